# revision 1
# baseline (speedup 1.0000x reference)
"""Trainium2 Bass kernel for 2-layer RGCN (mean aggregation) on 8 NeuronCores.

Design:
  - dst-sharded: core k owns destination rows [k*6250, (k+1)*6250), padded to
    6272 = 49 tiles of 128 rows. Each core computes its output rows entirely,
    so no cross-core reduction is needed.
  - Edges are grouped on the host by (core, relation, dst_tile); each group is
    padded to 3 chunks of 128 edge slots (max observed occupancy ~334 < 384).
  - Per chunk: indirect-DMA gather of the 128 source rows (bf16), a one-hot
    mask [128e, 128d] built on DVE from iota/is_equal, and a TensorE matmul
    mask^T @ msgs accumulated in PSUM -> per-(r, tile) segment sums.
  - Mean normalization (1/cnt per (r, dst)) is folded into the PSUM->SBUF copy
    as a per-partition scale on the Scalar engine.
  - The per-relation transform (agg @ W_r summed over r) runs on TensorE using
    PE-transposed aggregates as the stationary operand; root term and bias are
    extra accumulating matmuls into the same PSUM tile.
  - Two launches: L1 produces h shards; the host concatenates them (pure data
    movement) and launch 2 consumes the full h for its gathers. This avoids
    on-device collectives entirely.
  - All matmul operands are bf16 (fp32 PSUM accumulation); validated end-to-end
    numerically at ~0.3% relative error vs the fp32 reference.
"""
import numpy as np
import ml_dtypes

N = 50000
E = 800000
R = 8
IN, HID, OUT = 512, 256, 512
NCORES = 8
SHARD = 6250
TILES = 52                 # padded tile count (49 real + 3 empty)
LTILES = 13                # tiles per launch (4 launches per layer)
PSH = TILES * 128          # 6656 padded rows per core
C = 3                      # chunks per (relation, dst-tile) group
NCH = R * TILES * C        # chunks per core
bf16 = ml_dtypes.bfloat16

_pending_trace = {"l1": None, "l2": None}


# ---------------------------------------------------------------------------
# Workarounds for this container's walrus build (single sync-wait per
# instruction) and missing NTFF profile hook under axon.
# ---------------------------------------------------------------------------
def _install_tilefix():
    import concourse.mybir as mybir
    import concourse.tile as tile_mod
    from concourse.vector_clock import ScopedClock

    if getattr(tile_mod.TileContext, "_rgcn_patched", False):
        return
    counter = [0]

    def split_multiwaits(nc):
        for f in nc.m.functions:
            for bb in f.blocks:
                out = []
                changed = False
                for inst in bb.instructions:
                    si = inst.sync_info
                    waits = list(si.on_wait) if si is not None else []
                    if len(waits) > 1:
                        changed = True
                        for w in waits[:-1]:
                            counter[0] += 1
                            nop = mybir.InstNoOp(
                                name=f"I-wsplit-{counter[0]}", ins=[], outs=[])
                            nop.engine = inst.engine
                            nop.sync_info = mybir.SyncInfo(
                                on_wait=[w], on_update=[])
                            nc.register_instruction(nop, overwrite=True)
                            out.append(nop)
                        si.on_wait = waits[-1:]
                    out.append(inst)
                if changed:
                    bb.instructions = out

    def patched_drain_and_barrier(self, tick_clock, wait_clock):
        nc = self.nc
        drain_inst = nc.sync.drain()
        wait_clock.add_sem_waits(
            drain_inst.ins, ScopedClock({None: tick_clock.global_clock}))
        nc.all_engine_barrier()
        assert self.sems is not None
        popped = nc._tile_sem_poison_stack.pop()
        assert popped is self._sem_poison
        nc.clear_and_free_semaphores(list(self.sems.allocated().values()))
        nc.all_engine_barrier()
        split_multiwaits(nc)

    tile_mod.TileContext._drain_and_barrier = patched_drain_and_barrier
    tile_mod.TileContext._rgcn_patched = True


def _install_ntff_hook():
    import sys, types
    if 'antenv.axon_hooks' in sys.modules:
        return
    try:
        try:
            from trn_agent_boot.trn_boot import _ntff_profile_via_ctypes
        except ImportError:
            sys.path.insert(0, '/root/.axon_site')
            from trn_agent_boot.trn_boot import _ntff_profile_via_ctypes
        hook = _ntff_profile_via_ctypes('/opt/axon/libaxon_pjrt.so')
    except Exception:
        return
    mod = types.ModuleType('antenv.axon_hooks')
    mod.get_axon_ntff_profile_hook = lambda: hook
    mod.set_axon_ntff_profile_hook = lambda h: None
    sys.modules['antenv.axon_hooks'] = mod


# ---------------------------------------------------------------------------
# Host preprocessing: edge grouping/padding + per-core index/mask/scale arrays
# ---------------------------------------------------------------------------
def _host_prep(src, dst, et):
    src = src.astype(np.int64)
    dst = dst.astype(np.int64)
    et = et.astype(np.int64)

    seg = et * N + dst
    cnt = np.bincount(seg, minlength=R * N).astype(np.float32)
    inv = np.where(cnt > 0, 1.0 / np.maximum(cnt, 1), 0.0).astype(np.float32)

    core_of = dst // SHARD
    dloc = dst - core_of * SHARD
    tile_of = dloc // 128
    dst_in_tile = (dloc % 128).astype(np.float32)

    pad_src = (src // SHARD) * PSH + (src % SHARD)  # index into padded h

    per_core = []
    for c in range(NCORES):
        eids = np.nonzero(core_of == c)[0]
        key = et[eids] * TILES + tile_of[eids]
        order = np.argsort(key, kind='stable')
        eids = eids[order]
        key = key[order]
        starts = np.searchsorted(key, np.arange(R * TILES))
        ends = np.searchsorted(key, np.arange(R * TILES) + 1)
        ns = ends - starts
        if ns.max() > C * 128:
            raise RuntimeError(f"group overflow: {ns.max()} > {C*128}")

        slot_src = np.zeros((R * TILES, C * 128), np.int32)
        slot_src2 = np.zeros((R * TILES, C * 128), np.int32)
        slot_dl = np.full((R * TILES, C * 128), -1.0, np.float32)
        for g in range(R * TILES):
            n = ns[g]
            e = eids[starts[g]:ends[g]]
            slot_src[g, :n] = src[e]
            slot_src2[g, :n] = pad_src[e]
            slot_dl[g, :n] = dst_in_tile[e]

        # chunk ch of group g -> column g*C + ch; slot j in chunk -> partition j
        def to_cols(a, dt):
            return np.ascontiguousarray(
                a.reshape(R * TILES * C, 128).T).astype(dt)

        inv_cols = np.zeros((128, R * TILES), np.float32)
        base = c * SHARD
        rows = base + (np.arange(TILES * 128) % (TILES * 128))
        for t in range(TILES):
            rr = base + t * 128 + np.arange(128)
            ok = rr < (c + 1) * SHARD
            rc = np.minimum(rr, N - 1)
            for r in range(R):
                inv_cols[:, r * TILES + t] = np.where(ok, inv[r * N + rc], 0.0)

        per_core.append(dict(
            idx1=to_cols(slot_src, np.int32),
            idx2=to_cols(slot_src2, np.int32),
            dstloc=to_cols(slot_dl, np.float32),
            invc=np.ascontiguousarray(inv_cols),
        ))
    return per_core


def _pack_weights(W, nchunk):
    # W [R, K, M] with K = nchunk*128 -> [128, R*nchunk*M], block (r, c) at
    # columns (r*nchunk + c)*M
    Rr, K, M = W.shape
    out = np.zeros((128, Rr * nchunk * M), bf16)
    for r in range(Rr):
        for c in range(nchunk):
            out[:, (r * nchunk + c) * M:(r * nchunk + c + 1) * M] = \
                W[r, c * 128:(c + 1) * 128, :].astype(bf16)
    return out


def _pack_single(Wm, nchunk):
    K, M = Wm.shape
    out = np.zeros((128, nchunk * M), bf16)
    for c in range(nchunk):
        out[:, c * M:(c + 1) * M] = Wm[c * 128:(c + 1) * 128, :].astype(bf16)
    return out


def _shard_T(xf, c, width, t0, base_stride=SHARD):
    # rows of core c for launch tiles [t0, t0+LTILES), transposed. The source
    # holds core c's rows at offset c*base_stride with SHARD valid rows.
    nch = width // 128
    base = c * base_stride
    lo = base + t0 * 128
    hi = min(base + SHARD, lo + LTILES * 128)
    nrows = max(0, hi - lo)
    blk = np.zeros((width, LTILES * 128), np.float32)
    if nrows > 0:
        blk[:, :nrows] = xf[lo:hi].T
    out = np.zeros((128, nch * LTILES * 128), bf16)
    W = LTILES * 128
    for cc in range(nch):
        out[:, cc * W:(cc + 1) * W] = blk[cc * 128:(cc + 1) * 128].astype(bf16)
    return out


def _slice_cols(a, t0, per_tile):
    # a [128, R*TILES*per_tile] grouped (r, tile) -> launch cols for tiles
    # [t0, t0+LTILES) of every relation, relaid out as (r, local_tile)
    cols = []
    for r in range(R):
        s = (r * TILES + t0) * per_tile
        cols.append(a[:, s:s + LTILES * per_tile])
    return np.ascontiguousarray(np.concatenate(cols, axis=1))


# ---------------------------------------------------------------------------
# Device kernel builders
# ---------------------------------------------------------------------------
def _build_layer(layer):
    import concourse.bass as bass
    import concourse.mybir as mybir
    from concourse.tile import TileContext

    F = IN if layer == 1 else HID        # message width
    H = HID if layer == 1 else OUT       # output width
    FC = F // 128                        # feature chunks (4 or 2)
    NSRC = N if layer == 1 else NCORES * PSH
    T = LTILES
    LNCH = R * T * C

    nc = bass.Bass()
    xsrc = nc.dram_tensor('xsrc', [NSRC, F], mybir.dt.bfloat16, kind='ExternalInput')
    xT = nc.dram_tensor('xT', [128, FC * T * 128], mybir.dt.bfloat16, kind='ExternalInput')
    Wsb = nc.dram_tensor('Wsb', [128, R * FC * H], mybir.dt.bfloat16, kind='ExternalInput')
    rootsb = nc.dram_tensor('rootsb', [128, FC * H], mybir.dt.bfloat16, kind='ExternalInput')
    brow = nc.dram_tensor('brow', [1, H], mybir.dt.bfloat16, kind='ExternalInput')
    idx = nc.dram_tensor('idx', [128, LNCH], mybir.dt.int32, kind='ExternalInput')
    dstloc = nc.dram_tensor('dstloc', [128, LNCH], mybir.dt.float32, kind='ExternalInput')
    invc = nc.dram_tensor('invc', [128, R * T], mybir.dt.float32, kind='ExternalInput')
    iota = nc.dram_tensor('iota', [128, 128], mybir.dt.bfloat16, kind='ExternalInput')
    ident = nc.dram_tensor('ident', [128, 128], mybir.dt.bfloat16, kind='ExternalInput')
    out_dt = mybir.dt.bfloat16 if layer == 1 else mybir.dt.float32
    yout = nc.dram_tensor('yout', [T * 128, H], out_dt, kind='ExternalOutput')

    with TileContext(nc) as tc:
        with tc.tile_pool(name='const', bufs=1) as cp, \
             tc.tile_pool(name='gather', bufs=6) as gp, \
             tc.tile_pool(name='masks', bufs=6) as mp, \
             tc.tile_pool(name='aggs', bufs=3) as ap_, \
             tc.tile_pool(name='aggts', bufs=3) as atp, \
             tc.tile_pool(name='hout', bufs=3) as hp, \
             tc.tile_pool(name='pagg', bufs=2, space='PSUM') as pagg, \
             tc.tile_pool(name='ptr', bufs=2, space='PSUM') as ptr, \
             tc.tile_pool(name='pout', bufs=2, space='PSUM') as pout:

            xT_sb = cp.tile([128, FC * T * 128], mybir.dt.bfloat16)
            nc.sync.dma_start(out=xT_sb[:], in_=xT[:])
            W_sb = cp.tile([128, R * FC * H], mybir.dt.bfloat16)
            nc.sync.dma_start(out=W_sb[:], in_=Wsb[:])
            root_sb = cp.tile([128, FC * H], mybir.dt.bfloat16)
            nc.sync.dma_start(out=root_sb[:], in_=rootsb[:])
            b_sb = cp.tile([1, H], mybir.dt.bfloat16)
            nc.sync.dma_start(out=b_sb[:], in_=brow[:])
            ones_sb = cp.tile([1, 128], mybir.dt.bfloat16)
            nc.vector.memset(ones_sb[:], 1.0)
            idx_sb = cp.tile([128, LNCH], mybir.dt.int32)
            nc.sync.dma_start(out=idx_sb[:], in_=idx[:])
            dl_sb = cp.tile([128, LNCH], mybir.dt.float32)
            nc.sync.dma_start(out=dl_sb[:], in_=dstloc[:])
            inv_sb = cp.tile([128, R * T], mybir.dt.float32)
            nc.sync.dma_start(out=inv_sb[:], in_=invc[:])
            iota_sb = cp.tile([128, 128], mybir.dt.bfloat16)
            nc.sync.dma_start(out=iota_sb[:], in_=iota[:])
            id_sb = cp.tile([128, 128], mybir.dt.bfloat16)
            nc.sync.dma_start(out=id_sb[:], in_=ident[:])

            for t in range(T):
                opsum = pout.tile([128, H], mybir.dt.float32)
                for r in range(R):
                    g = r * T + t
                    agg = pagg.tile([128, F], mybir.dt.float32)
                    for ch in range(C):
                        col = g * C + ch
                        msgs = gp.tile([128, F], mybir.dt.bfloat16, tag='msgs')
                        nc.gpsimd.indirect_dma_start(
                            out=msgs[:], out_offset=None, in_=xsrc[:],
                            in_offset=bass.IndirectOffsetOnAxis(
                                ap=idx_sb[:, col:col + 1], axis=0))
                        mask = mp.tile([128, 128], mybir.dt.bfloat16, tag='mask')
                        nc.vector.tensor_scalar(
                            out=mask[:], in0=iota_sb[:],
                            scalar1=dl_sb[:, col:col + 1], scalar2=None,
                            op0=mybir.AluOpType.is_equal)
                        nc.tensor.matmul(
                            out=agg[:], lhsT=mask[:], rhs=msgs[:],
                            start=(ch == 0), stop=(ch == C - 1))
                    # scale by 1/cnt (per dst row) while copying PSUM->SBUF
                    agg_s = ap_.tile([128, F], mybir.dt.bfloat16, tag='aggs')
                    nc.scalar.activation(
                        out=agg_s[:], in_=agg[:],
                        func=mybir.ActivationFunctionType.Copy,
                        scale=inv_sb[:, g:g + 1])
                    # transpose agg_s -> aggT (feature-major) via TensorE
                    trp = ptr.tile([128, F], mybir.dt.bfloat16, tag='trp')
                    for c2 in range(FC):
                        nc.tensor.transpose(
                            out=trp[:, c2 * 128:(c2 + 1) * 128],
                            in_=agg_s[:, c2 * 128:(c2 + 1) * 128],
                            identity=id_sb[:])
                    aggT = atp.tile([128, F], mybir.dt.bfloat16, tag='aggT')
                    nc.vector.tensor_copy(out=aggT[:], in_=trp[:])
                    # transform: opsum += agg @ W_r
                    for c2 in range(FC):
                        nc.tensor.matmul(
                            out=opsum[:],
                            lhsT=aggT[:, c2 * 128:(c2 + 1) * 128],
                            rhs=W_sb[:, (r * FC + c2) * H:(r * FC + c2 + 1) * H],
                            start=(r == 0 and c2 == 0), stop=False)
                # root term + bias
                for c2 in range(FC):
                    nc.tensor.matmul(
                        out=opsum[:],
                        lhsT=xT_sb[:, (c2 * T + t) * 128:(c2 * T + t + 1) * 128],
                        rhs=root_sb[:, c2 * H:(c2 + 1) * H],
                        start=False, stop=False)
                nc.tensor.matmul(
                    out=opsum[:], lhsT=ones_sb[:], rhs=b_sb[:],
                    start=False, stop=True)

                if layer == 1:
                    h_t = hp.tile([128, H], mybir.dt.bfloat16, tag='ht')
                    nc.scalar.activation(
                        out=h_t[:], in_=opsum[:],
                        func=mybir.ActivationFunctionType.Relu)
                    nc.sync.dma_start(
                        out=yout[t * 128:(t + 1) * 128, :], in_=h_t[:])
                else:
                    nrm2 = hp.tile([128, 1], mybir.dt.float32, tag='n2')
                    sq = hp.tile([128, OUT], mybir.dt.float32, tag='sq')
                    nc.scalar.activation(
                        out=sq[:], in_=opsum[:],
                        func=mybir.ActivationFunctionType.Square,
                        accum_out=nrm2[:])
                    srt = hp.tile([128, 1], mybir.dt.float32, tag='srt')
                    nc.scalar.activation(
                        out=srt[:], in_=nrm2[:],
                        func=mybir.ActivationFunctionType.Sqrt)
                    nc.vector.tensor_scalar_max(srt[:], srt[:], 1e-12)
                    rcp = hp.tile([128, 1], mybir.dt.float32, tag='rcp')
                    nc.vector.reciprocal(rcp[:], srt[:])
                    o_t = hp.tile([128, OUT], mybir.dt.float32, tag='ot')
                    nc.scalar.activation(
                        out=o_t[:], in_=opsum[:],
                        func=mybir.ActivationFunctionType.Copy,
                        scale=rcp[:])
                    nc.sync.dma_start(
                        out=yout[t * 128:(t + 1) * 128, :], in_=o_t[:])
    return nc


def _run(nc, in_maps, trace=False):
    from concourse import bass_utils
    res = bass_utils.run_bass_kernel_spmd(
        nc, in_maps, core_ids=list(range(NCORES)), trace=trace)
    return res


# ---------------------------------------------------------------------------
# Entry point
# ---------------------------------------------------------------------------
def kernel(x, W1, root1, b1, W2, root2, b2, src, dst, edge_type,
           _trace=None):
    global C, NCH
    _install_tilefix()
    _install_ntff_hook()

    # size chunk capacity to the actual densest (relation, dst-tile) group
    _d = np.asarray(dst).astype(np.int64)
    _e = np.asarray(edge_type).astype(np.int64)
    _g = ((_d // SHARD) * R + _e) * TILES + (_d % SHARD) // 128
    _mx = int(np.bincount(_g, minlength=NCORES * R * TILES).max())
    C = max(3, -(-_mx // 128))
    NCH = R * TILES * C

    x = np.asarray(x, np.float32)
    per_core = _host_prep(np.asarray(src), np.asarray(dst),
                          np.asarray(edge_type))

    iota_np = np.broadcast_to(np.arange(128, dtype=np.float32),
                              (128, 128)).astype(bf16)
    ident_np = np.eye(128, dtype=np.float32).astype(bf16)

    x_bf = x.astype(bf16)
    W1p = _pack_weights(np.asarray(W1, np.float32), IN // 128)
    r1p = _pack_single(np.asarray(root1, np.float32), IN // 128)
    b1p = np.asarray(b1, np.float32)[None, :].astype(bf16)
    W2p = _pack_weights(np.asarray(W2, np.float32), HID // 128)
    r2p = _pack_single(np.asarray(root2, np.float32), HID // 128)
    b2p = np.asarray(b2, np.float32)[None, :].astype(bf16)

    # ---- layer 1: 4 launches of LTILES tiles each ----
    nc1 = _build_layer(1)
    h_full = np.zeros((NCORES * PSH, HID), bf16)
    t_l1 = 0
    for li in range(TILES // LTILES):
        t0 = li * LTILES
        in_maps1 = []
        for c in range(NCORES):
            pc = per_core[c]
            in_maps1.append(dict(
                xsrc=x_bf, xT=_shard_T(x, c, IN, t0), Wsb=W1p, rootsb=r1p,
                brow=b1p, idx=_slice_cols(pc['idx1'], t0, C),
                dstloc=_slice_cols(pc['dstloc'], t0, C),
                invc=_slice_cols(pc['invc'], t0, 1),
                iota=iota_np, ident=ident_np))
        res1 = _run(nc1, in_maps1, trace=(_trace == f'l1_{li}'))
        if res1.exec_time_ns:
            t_l1 += res1.exec_time_ns
        for c in range(NCORES):
            h_full[c * PSH + t0 * 128: c * PSH + (t0 + LTILES) * 128] = \
                res1.results[c]['yout']
    _pending_trace['l1'] = t_l1 or None

    # ---- layer 2: 4 launches ----
    h_f32 = h_full.astype(np.float32)
    nc2 = _build_layer(2)
    out = np.empty((N, OUT), np.float32)
    t_l2 = 0
    for li in range(TILES // LTILES):
        t0 = li * LTILES
        in_maps2 = []
        for c in range(NCORES):
            pc = per_core[c]
            in_maps2.append(dict(
                xsrc=h_full, xT=_shard_T(h_f32, c, HID, t0, PSH), Wsb=W2p,
                rootsb=r2p, brow=b2p, idx=_slice_cols(pc['idx2'], t0, C),
                dstloc=_slice_cols(pc['dstloc'], t0, C),
                invc=_slice_cols(pc['invc'], t0, 1),
                iota=iota_np, ident=ident_np))
        res2 = _run(nc2, in_maps2, trace=(_trace == f'l2_{li}'))
        if res2.exec_time_ns:
            t_l2 += res2.exec_time_ns
        for c in range(NCORES):
            rows0 = c * SHARD + t0 * 128
            nrows = max(0, min((c + 1) * SHARD, rows0 + LTILES * 128) - rows0)
            if nrows > 0:
                out[rows0:rows0 + nrows] = \
                    res2.results[c]['yout'][:nrows].astype(np.float32)
    _pending_trace['l2'] = t_l2 or None
    return out



# revision 9
# speedup vs baseline: 1.2003x; 1.2003x over previous
"""Trainium2 Bass kernel for 2-layer RGCN (mean aggregation) on 8 NeuronCores.

v2 design:
  - dst-sharded: core k owns destination rows [k*6250, (k+1)*6250) = 49 tiles
    of 128. Each core computes its output rows entirely; no collectives.
  - Gathers use the batched ant dma_gather (Q7 ucode): ~1us fixed cost per
    call amortized over 1024 rows, vs ~1.1us per 128-row indirect DMA in v1.
    Indices are int16, so sources are split into two classes by row range
    (src < 32768 gathers from the table base, src >= 32768 from a +32768 row
    offset); chunks are class-pure.
  - Edges grouped by (dst_tile, relation) into 128-slot chunks; per-group
    chunk counts are the max over cores so one compiled program (SPMD) fits
    all 8 cores; per-core data (indices, in-tile dst positions, 1/cnt) fills
    the uniform slots, padded slots gather row 0 and mask to zero.
  - Aggregation produces aggT = msgs^T-scatter directly: per chunk the
    gathered messages are the matmul stationary operand and the one-hot
    dst mask (built on DVE, 1/cnt mean scaling folded in) is the moving
    operand, accumulating agg^T[f, dst] in PSUM. This removes the per-group
    TensorE transposes and PSUM round-trips of v1.
  - Transform: per (tile, relation): aggT chunks (stationary) x W_r -> opsum
    [dst, H] accumulated in PSUM along with x@root and bias.
  - One launch per layer; layer 1 output rows return to the host, which
    assembles the full h table (pure data movement) for layer 2's gathers.
  - bf16 operands with fp32 PSUM accumulation (~0.3-0.4% rel error).
"""
import numpy as np
import ml_dtypes

N = 50000
E = 800000
R = 8
IN, HID, OUT = 512, 256, 512
NCORES = 8
SHARD = 6250
TILES = 49                 # ceil(6250/128)
NG = TILES * R             # groups per core; gid = t*R + r
SPLIT = 32768              # int16-safe gather index split
BATCH = 8                  # chunks per dma_gather call
bf16 = ml_dtypes.bfloat16

_pending_trace = {"l1": None, "l2": None}
_last_traced = [None]


# ---------------------------------------------------------------------------
# Workarounds for this container's walrus build (single sync-wait per
# instruction) and missing NTFF profile hook under axon.
# ---------------------------------------------------------------------------
def _install_tilefix():
    import concourse.mybir as mybir
    import concourse.tile as tile_mod
    from concourse.vector_clock import ScopedClock

    if getattr(tile_mod.TileContext, "_rgcn_patched", False):
        return
    counter = [0]

    def split_multiwaits(nc):
        for f in nc.m.functions:
            for bb in f.blocks:
                out = []
                changed = False
                for inst in bb.instructions:
                    si = inst.sync_info
                    waits = list(si.on_wait) if si is not None else []
                    if len(waits) > 1:
                        changed = True
                        for w in waits[:-1]:
                            counter[0] += 1
                            nop = mybir.InstNoOp(
                                name=f"I-wsplit-{counter[0]}", ins=[], outs=[])
                            nop.engine = inst.engine
                            nop.sync_info = mybir.SyncInfo(
                                on_wait=[w], on_update=[])
                            nc.register_instruction(nop, overwrite=True)
                            out.append(nop)
                        si.on_wait = waits[-1:]
                    out.append(inst)
                if changed:
                    bb.instructions = out

    def patched_drain_and_barrier(self, tick_clock, wait_clock):
        nc = self.nc
        drain_inst = nc.sync.drain()
        wait_clock.add_sem_waits(
            drain_inst.ins, ScopedClock({None: tick_clock.global_clock}))
        nc.all_engine_barrier()
        assert self.sems is not None
        popped = nc._tile_sem_poison_stack.pop()
        assert popped is self._sem_poison
        nc.clear_and_free_semaphores(list(self.sems.allocated().values()))
        nc.all_engine_barrier()
        split_multiwaits(nc)

    tile_mod.TileContext._drain_and_barrier = patched_drain_and_barrier
    tile_mod.TileContext._rgcn_patched = True


def _install_ntff_hook():
    import sys, types
    if 'antenv.axon_hooks' in sys.modules:
        return
    try:
        try:
            from trn_agent_boot.trn_boot import _ntff_profile_via_ctypes
        except ImportError:
            sys.path.insert(0, '/root/.axon_site')
            from trn_agent_boot.trn_boot import _ntff_profile_via_ctypes
        hook = _ntff_profile_via_ctypes('/opt/axon/libaxon_pjrt.so')
    except Exception:
        return
    mod = types.ModuleType('antenv.axon_hooks')
    mod.get_axon_ntff_profile_hook = lambda: hook
    mod.set_axon_ntff_profile_hook = lambda h: None
    sys.modules['antenv.axon_hooks'] = mod


# ---------------------------------------------------------------------------
# Host preprocessing: uniform chunk schedule + per-core slot data
# ---------------------------------------------------------------------------
def _host_prep(src, dst, et):
    src = np.asarray(src).astype(np.int64)
    dst = np.asarray(dst).astype(np.int64)
    et = np.asarray(et).astype(np.int64)

    seg = et * N + dst
    segcnt = np.bincount(seg, minlength=R * N)
    inv_seg = np.where(segcnt > 0, 1.0 / np.maximum(segcnt, 1),
                       0.0).astype(np.float32)

    core = dst // SHARD
    dloc = dst - core * SHARD
    tl = dloc // 128
    dl = (dloc % 128).astype(np.float32)
    gid = tl * R + et
    cls = (src >= SPLIT).astype(np.int64)

    bucket = (core * NG + gid) * 2 + cls
    cnts = np.bincount(bucket, minlength=NCORES * NG * 2).reshape(
        NCORES, NG, 2)
    C = -(-cnts.max(axis=0) // 128)          # [NG, 2] uniform chunk counts

    ch0 = np.concatenate([[0], np.cumsum(C.sum(axis=1))])[:-1]  # [NG]
    base_pos = np.zeros((NG, 2), np.int64)
    base_pos[:, 0] = np.cumsum(C[:, 0]) - C[:, 0]
    base_pos[:, 1] = np.cumsum(C[:, 1]) - C[:, 1]
    NCH = int(C.sum())
    Scls = [int(C[:, 0].sum()), int(C[:, 1].sum())]

    order = np.argsort(bucket, kind='stable')
    sk = bucket[order]
    nb = NCORES * NG * 2
    starts = np.searchsorted(sk, np.arange(nb))
    ends = np.searchsorted(sk, np.arange(nb) + 1)

    invv = inv_seg[seg]
    per_core = []
    for c in range(NCORES):
        idx_cls = [np.zeros(max(Scls[0], 1) * 128, np.int32),
                   np.zeros(max(Scls[1], 1) * 128, np.int32)]
        dl_arr = np.full((128, NCH), -1.0, np.float32)
        inv_arr = np.zeros((128, NCH), np.float32)
        for g in range(NG):
            for cl in (0, 1):
                b = ((c * NG + g) * 2 + cl)
                e = order[starts[b]:ends[b]]
                n = len(e)
                if n == 0:
                    continue
                pos0 = base_pos[g, cl] * 128
                idx_cls[cl][pos0:pos0 + n] = src[e] - (SPLIT if cl else 0)
                ar = np.arange(n)
                chs = ch0[g] + (C[g, 0] if cl else 0) + ar // 128
                parts = ar % 128
                dl_arr[parts, chs] = dl[e]
                inv_arr[parts, chs] = invv[e]
        wrapped = []
        for cl in (0, 1):
            nbatch = max(1, -(-Scls[cl] // BATCH))
            tot = nbatch * BATCH * 128
            a = np.zeros(tot, np.int32)
            a[:len(idx_cls[cl])] = idx_cls[cl]
            w = a.reshape(-1, 16).T.astype(np.int16)   # [16, tot/16]
            wrapped.append(np.ascontiguousarray(np.tile(w, (8, 1))))
        per_core.append(dict(idx_lo=wrapped[0], idx_hi=wrapped[1],
                             dl=dl_arr, inv=inv_arr))

    gid_of_pos = [np.repeat(np.arange(NG), C[:, 0]),
                  np.repeat(np.arange(NG), C[:, 1])]
    sched = dict(C=C, NCH=NCH, ch0=ch0, base_pos=base_pos, Scls=Scls,
                 gid_of_pos=gid_of_pos)
    return sched, per_core


def _pack_weights(W, nchunk, H):
    Rr = W.shape[0]
    out = np.zeros((128, Rr * nchunk * H), bf16)
    for r in range(Rr):
        for c in range(nchunk):
            out[:, (r * nchunk + c) * H:(r * nchunk + c + 1) * H] = \
                W[r, c * 128:(c + 1) * 128, :].astype(bf16)
    return out


def _pack_single(Wm, nchunk, H):
    out = np.zeros((128, nchunk * H), bf16)
    for c in range(nchunk):
        out[:, c * H:(c + 1) * H] = Wm[c * 128:(c + 1) * 128, :].astype(bf16)
    return out


def _make_xT(xf, c, width):
    # rows of core c transposed, tiled: col block (t*FC + fc)*128
    FC = width // 128
    out = np.zeros((128, TILES * FC * 128), bf16)
    base = c * SHARD
    nrows = min(SHARD, TILES * 128)
    blk = np.zeros((width, TILES * 128), np.float32)
    blk[:, :nrows] = xf[base:base + nrows].T
    for t in range(TILES):
        for fc in range(FC):
            out[:, (t * FC + fc) * 128:(t * FC + fc + 1) * 128] = \
                blk[fc * 128:(fc + 1) * 128,
                    t * 128:(t + 1) * 128].astype(bf16)
    return out


# ---------------------------------------------------------------------------
# Device kernel builder (one launch per layer)
# ---------------------------------------------------------------------------
def _build_layer(layer, sched):
    import concourse.bass as bass
    import concourse.mybir as mybir
    from concourse.tile import TileContext
    from concourse.library_config import mlp
    from concourse.library_overlay import lower_extended_insts

    F = IN if layer == 1 else HID
    H = HID if layer == 1 else OUT
    FC = F // 128
    C = sched['C']
    NCH = sched['NCH']
    ch0 = sched['ch0']
    base_pos = sched['base_pos']
    Scls = sched['Scls']
    gid_of_pos = sched['gid_of_pos']
    NB = [max(1, -(-Scls[0] // BATCH)), max(1, -(-Scls[1] // BATCH))]
    # first tile that consumes each batch
    first_tile = []
    for cl in (0, 1):
        ft = []
        for k in range(NB[cl]):
            p = k * BATCH
            g = gid_of_pos[cl][p] if p < Scls[cl] else NG - 1
            ft.append(int(g) // R)
        first_tile.append(ft)

    nc = bass.Bass()
    dt = mybir.dt
    xsrc = nc.dram_tensor('xsrc', [N, F], dt.bfloat16, kind='ExternalInput')
    idx_lo = nc.dram_tensor('idx_lo', [128, NB[0] * BATCH * 8], dt.int16,
                            kind='ExternalInput')
    idx_hi = nc.dram_tensor('idx_hi', [128, NB[1] * BATCH * 8], dt.int16,
                            kind='ExternalInput')
    dlt = nc.dram_tensor('dlt', [128, NCH], dt.float32, kind='ExternalInput')
    invt = nc.dram_tensor('invt', [128, NCH], dt.float32,
                          kind='ExternalInput')
    iota = nc.dram_tensor('iota', [128, 128], dt.bfloat16,
                          kind='ExternalInput')
    Wsb = nc.dram_tensor('Wsb', [128, R * FC * H], dt.bfloat16,
                         kind='ExternalInput')
    rootsb = nc.dram_tensor('rootsb', [128, FC * H], dt.bfloat16,
                            kind='ExternalInput')
    brow = nc.dram_tensor('brow', [1, H], dt.bfloat16, kind='ExternalInput')
    xT = nc.dram_tensor('xT', [128, TILES * FC * 128], dt.bfloat16,
                        kind='ExternalInput')
    out_dt = dt.bfloat16 if layer == 1 else dt.float32
    yout = nc.dram_tensor('yout', [TILES * 128, H], out_dt,
                          kind='ExternalOutput')

    is_equal = mybir.AluOpType.is_equal
    mult = mybir.AluOpType.mult

    with TileContext(nc) as tc:
        with tc.tile_pool(name='const', bufs=1) as cp, \
             tc.tile_pool(name='glo', bufs=3) as glo, \
             tc.tile_pool(name='ghi', bufs=3) as ghi, \
             tc.tile_pool(name='xtp', bufs=2) as xtp, \
             tc.tile_pool(name='masks', bufs=6) as mp, \
             tc.tile_pool(name='asb', bufs=4) as asb, \
             tc.tile_pool(name='hout', bufs=3) as hp, \
             tc.tile_pool(name='pagg', bufs=3, space='PSUM') as pagg, \
             tc.tile_pool(name='pout', bufs=2, space='PSUM') as pout:

            nc.gpsimd.load_library(mlp)

            il_sb = cp.tile([128, NB[0] * BATCH * 8], dt.int16)
            nc.sync.dma_start(out=il_sb[:], in_=idx_lo[:])
            ih_sb = cp.tile([128, NB[1] * BATCH * 8], dt.int16)
            nc.sync.dma_start(out=ih_sb[:], in_=idx_hi[:])
            dl_sb = cp.tile([128, NCH], dt.float32)
            nc.sync.dma_start(out=dl_sb[:], in_=dlt[:])
            inv_sb = cp.tile([128, NCH], dt.float32)
            nc.sync.dma_start(out=inv_sb[:], in_=invt[:])
            iota_sb = cp.tile([128, 128], dt.bfloat16)
            nc.sync.dma_start(out=iota_sb[:], in_=iota[:])
            W_sb = cp.tile([128, R * FC * H], dt.bfloat16)
            nc.sync.dma_start(out=W_sb[:], in_=Wsb[:])
            root_sb = cp.tile([128, FC * H], dt.bfloat16)
            nc.sync.dma_start(out=root_sb[:], in_=rootsb[:])
            b_sb = cp.tile([1, H], dt.bfloat16)
            nc.sync.dma_start(out=b_sb[:], in_=brow[:])
            ones_sb = cp.tile([1, 128], dt.bfloat16)
            nc.vector.memset(ones_sb[:], 1.0)

            idx_sbs = [il_sb, ih_sb]
            srcs = [xsrc[:, :], xsrc[SPLIT:, :]]
            gpools = [glo, ghi]
            gtiles = [[None] * NB[0], [None] * NB[1]]
            next_b = [0, 0]
            nidx_reg = nc.gpsimd.to_reg(BATCH * 128)

            def issue_up_to(tile_idx):
                for cl in (0, 1):
                    while (next_b[cl] < NB[cl]
                           and first_tile[cl][next_b[cl]] <= tile_idx):
                        k = next_b[cl]
                        gt = gpools[cl].tile([128, BATCH, F], dt.bfloat16,
                                             tag=f'g{cl}')
                        nidx = BATCH * 128
                        nc.gpsimd.dma_gather(
                            gt[:, :, :], srcs[cl],
                            idx_sbs[cl][:, k * (nidx // 16):
                                        (k + 1) * (nidx // 16)],
                            nidx, nidx_reg, F,
                            single_packet=(nidx <= 128))
                        gtiles[cl][k] = gt
                        next_b[cl] += 1

            copy_rr = [0]
            for t in range(TILES):
                issue_up_to(t + 1)
                opsum = pout.tile([128, 512], dt.float32, tag='o')
                xT_t = xtp.tile([128, FC * 128], dt.bfloat16, tag='xT')
                nc.sync.dma_start(
                    out=xT_t[:],
                    in_=xT[:, t * FC * 128:(t + 1) * FC * 128])
                for fc in range(FC):
                    nc.tensor.matmul(
                        out=opsum[:, :H],
                        lhsT=xT_t[:, fc * 128:(fc + 1) * 128],
                        rhs=root_sb[:, fc * H:(fc + 1) * H],
                        start=(fc == 0), stop=False)
                for r in range(R):
                    g = t * R + r
                    nch_g = int(C[g, 0] + C[g, 1])
                    if nch_g == 0:
                        continue
                    apsum = pagg.tile([128, 512], dt.float32, tag='a')
                    chidx = 0
                    for cl in (0, 1):
                        for k in range(int(C[g, cl])):
                            ch = int(ch0[g]) + (int(C[g, 0]) if cl else 0) + k
                            p = int(base_pos[g, cl]) + k
                            bt = gtiles[cl][p // BATCH]
                            col = p % BATCH
                            m = mp.tile([128, 128], dt.bfloat16, tag='m')
                            nc.vector.tensor_scalar(
                                out=m[:], in0=iota_sb[:],
                                scalar1=dl_sb[:, ch:ch + 1],
                                scalar2=inv_sb[:, ch:ch + 1],
                                op0=is_equal, op1=mult)
                            for fc in range(FC):
                                nc.tensor.matmul(
                                    out=apsum[:, fc * 128:(fc + 1) * 128],
                                    lhsT=bt[:, col,
                                            fc * 128:(fc + 1) * 128],
                                    rhs=m[:],
                                    start=(chidx == 0 and fc == 0),
                                    stop=(chidx == nch_g - 1
                                          and fc == FC - 1))
                            chidx += 1
                    aggT = asb.tile([128, FC * 128], dt.bfloat16, tag='at')
                    if copy_rr[0] % 4 == 3:
                        nc.vector.tensor_copy(out=aggT[:],
                                              in_=apsum[:, :FC * 128])
                    else:
                        nc.scalar.activation(
                            out=aggT[:], in_=apsum[:, :FC * 128],
                            func=mybir.ActivationFunctionType.Copy)
                    copy_rr[0] += 1
                    for fc in range(FC):
                        nc.tensor.matmul(
                            out=opsum[:, :H],
                            lhsT=aggT[:, fc * 128:(fc + 1) * 128],
                            rhs=W_sb[:, (r * FC + fc) * H:
                                     (r * FC + fc + 1) * H],
                            start=False, stop=False)
                nc.tensor.matmul(
                    out=opsum[:, :H], lhsT=ones_sb[:], rhs=b_sb[:],
                    start=False, stop=True)

                if layer == 1:
                    h_t = hp.tile([128, H], dt.bfloat16, tag='ht')
                    nc.scalar.activation(
                        out=h_t[:], in_=opsum[:, :H],
                        func=mybir.ActivationFunctionType.Relu)
                    nc.sync.dma_start(
                        out=yout[t * 128:(t + 1) * 128, :], in_=h_t[:])
                else:
                    nrm2 = hp.tile([128, 1], dt.float32, tag='n2')
                    sq = hp.tile([128, OUT], dt.float32, tag='sq')
                    nc.scalar.activation(
                        out=sq[:], in_=opsum[:, :H],
                        func=mybir.ActivationFunctionType.Square,
                        accum_out=nrm2[:])
                    srt = hp.tile([128, 1], dt.float32, tag='srt')
                    nc.scalar.activation(
                        out=srt[:], in_=nrm2[:],
                        func=mybir.ActivationFunctionType.Sqrt)
                    nc.vector.tensor_scalar_max(srt[:], srt[:], 1e-12)
                    rcp = hp.tile([128, 1], dt.float32, tag='rcp')
                    nc.vector.reciprocal(rcp[:], srt[:])
                    o_t = hp.tile([128, OUT], dt.float32, tag='ot')
                    nc.scalar.activation(
                        out=o_t[:], in_=opsum[:, :H],
                        func=mybir.ActivationFunctionType.Copy,
                        scale=rcp[:])
                    nc.sync.dma_start(
                        out=yout[t * 128:(t + 1) * 128, :], in_=o_t[:])
    lower_extended_insts(nc)
    return nc


def _run(nc, in_maps, trace=False):
    from concourse import bass_utils
    res = bass_utils.run_bass_kernel_spmd(
        nc, in_maps, core_ids=list(range(NCORES)), trace=trace)
    if trace:
        _last_traced[0] = res
    return res


# ---------------------------------------------------------------------------
# Entry point
# ---------------------------------------------------------------------------
def kernel(x, W1, root1, b1, W2, root2, b2, src, dst, edge_type,
           _trace=None):
    _install_tilefix()
    _install_ntff_hook()

    x = np.asarray(x, np.float32)
    sched, per_core = _host_prep(src, dst, edge_type)

    iota_np = np.broadcast_to(np.arange(128, dtype=np.float32),
                              (128, 128)).astype(bf16)
    x_bf = np.ascontiguousarray(x.astype(bf16))
    W1p = _pack_weights(np.asarray(W1, np.float32), IN // 128, HID)
    r1p = _pack_single(np.asarray(root1, np.float32), IN // 128, HID)
    b1p = np.asarray(b1, np.float32)[None, :].astype(bf16)
    W2p = _pack_weights(np.asarray(W2, np.float32), HID // 128, OUT)
    r2p = _pack_single(np.asarray(root2, np.float32), HID // 128, OUT)
    b2p = np.asarray(b2, np.float32)[None, :].astype(bf16)

    # ---- layer 1 ----
    nc1 = _build_layer(1, sched)
    in_maps1 = []
    for c in range(NCORES):
        pc = per_core[c]
        in_maps1.append(dict(
            xsrc=x_bf, idx_lo=pc['idx_lo'], idx_hi=pc['idx_hi'],
            dlt=pc['dl'], invt=pc['inv'], iota=iota_np, Wsb=W1p,
            rootsb=r1p, brow=b1p, xT=_make_xT(x, c, IN)))
    res1 = _run(nc1, in_maps1, trace=(_trace in ('l1', 'l1_0')))
    _pending_trace['l1'] = res1.exec_time_ns

    h_tab = np.empty((N, HID), bf16)
    for c in range(NCORES):
        h_tab[c * SHARD:(c + 1) * SHARD] = res1.results[c]['yout'][:SHARD]
    h_f32 = h_tab.astype(np.float32)

    # ---- layer 2 ----
    nc2 = _build_layer(2, sched)
    in_maps2 = []
    for c in range(NCORES):
        pc = per_core[c]
        in_maps2.append(dict(
            xsrc=h_tab, idx_lo=pc['idx_lo'], idx_hi=pc['idx_hi'],
            dlt=pc['dl'], invt=pc['inv'], iota=iota_np, Wsb=W2p,
            rootsb=r2p, brow=b2p, xT=_make_xT(h_f32, c, HID)))
    res2 = _run(nc2, in_maps2, trace=(_trace in ('l2', 'l2_0')))
    _pending_trace['l2'] = res2.exec_time_ns

    out = np.empty((N, OUT), np.float32)
    for c in range(NCORES):
        out[c * SHARD:(c + 1) * SHARD] = \
            res2.results[c]['yout'][:SHARD].astype(np.float32)
    return out


# revision 16
# speedup vs baseline: 1.5846x; 1.3202x over previous
"""Trainium2 Bass kernel for 2-layer RGCN (mean aggregation) on 8 NeuronCores.

v3 design:
  - dst-sharded: core k owns destination rows [k*6250, (k+1)*6250) = 49 tiles
    of 128. Each core computes its output rows entirely; no collectives.
  - Gathers use the batched ant dma_gather (Q7 ucode). The HW bound is
    ~9.5ns per gathered row (SDMA per-descriptor pipeline), so the schedule
    minimizes gathered slots: per (dst_tile, relation, class) the slot count
    is the max edge count over the 8 cores (one SPMD program fits all), and
    slots pack densely into 128-row chunks that may straddle group
    boundaries (each straddled chunk does one extra masked matmul per
    feature chunk instead of padding the gather).
  - int16 gather indices: sources split into two classes by row range
    (src < 32768 from the table base, src >= 32768 from a +32768 offset).
  - Aggregation produces aggT directly: gathered messages are the matmul
    stationary operand, the one-hot dst mask (DVE-built, 1/cnt folded in)
    is the moving operand, accumulating agg^T[f, dst] per group in PSUM.
  - Transform per (tile, relation): aggT chunks x W_r -> opsum [dst, H]
    in PSUM along with x@root and bias.
  - One launch per layer; layer 1 output returns to the host, which
    assembles the h table (pure data movement) for layer 2's gathers.
  - bf16 operands, fp32 PSUM accumulation (~0.3% rel error).
"""
import numpy as np
import ml_dtypes

N = 50000
E = 800000
R = 8
IN, HID, OUT = 512, 256, 512
NCORES = 8
SHARD = 6250
TILES = 49                 # ceil(6250/128)
NG = TILES * R             # groups per core; gid = t*R + r
SPLIT = 32768              # int16-safe gather index split
BATCH = 8                  # chunks per dma_gather call
MAXSEG = 8                 # max group-segments per chunk (iota width)
bf16 = ml_dtypes.bfloat16

_pending_trace = {"l1": None, "l2": None}
_last_traced = [None]


# ---------------------------------------------------------------------------
# Workarounds for this container's walrus build (single sync-wait per
# instruction) and missing NTFF profile hook under axon.
# ---------------------------------------------------------------------------
def _install_tilefix():
    import concourse.mybir as mybir
    import concourse.tile as tile_mod
    from concourse.vector_clock import ScopedClock

    if getattr(tile_mod.TileContext, "_rgcn_patched", False):
        return
    counter = [0]

    def split_multiwaits(nc):
        for f in nc.m.functions:
            for bb in f.blocks:
                out = []
                changed = False
                for inst in bb.instructions:
                    si = inst.sync_info
                    waits = list(si.on_wait) if si is not None else []
                    if len(waits) > 1:
                        changed = True
                        for w in waits[:-1]:
                            counter[0] += 1
                            nop = mybir.InstNoOp(
                                name=f"I-wsplit-{counter[0]}", ins=[], outs=[])
                            nop.engine = inst.engine
                            nop.sync_info = mybir.SyncInfo(
                                on_wait=[w], on_update=[])
                            nc.register_instruction(nop, overwrite=True)
                            out.append(nop)
                        si.on_wait = waits[-1:]
                    out.append(inst)
                if changed:
                    bb.instructions = out

    def patched_drain_and_barrier(self, tick_clock, wait_clock):
        nc = self.nc
        drain_inst = nc.sync.drain()
        wait_clock.add_sem_waits(
            drain_inst.ins, ScopedClock({None: tick_clock.global_clock}))
        nc.all_engine_barrier()
        assert self.sems is not None
        popped = nc._tile_sem_poison_stack.pop()
        assert popped is self._sem_poison
        nc.clear_and_free_semaphores(list(self.sems.allocated().values()))
        nc.all_engine_barrier()
        split_multiwaits(nc)

    tile_mod.TileContext._drain_and_barrier = patched_drain_and_barrier
    tile_mod.TileContext._rgcn_patched = True


def _install_ntff_hook():
    import sys, types
    if 'antenv.axon_hooks' in sys.modules:
        return
    try:
        try:
            from trn_agent_boot.trn_boot import _ntff_profile_via_ctypes
        except ImportError:
            sys.path.insert(0, '/root/.axon_site')
            from trn_agent_boot.trn_boot import _ntff_profile_via_ctypes
        hook = _ntff_profile_via_ctypes('/opt/axon/libaxon_pjrt.so')
    except Exception:
        return
    mod = types.ModuleType('antenv.axon_hooks')
    mod.get_axon_ntff_profile_hook = lambda: hook
    mod.set_axon_ntff_profile_hook = lambda h: None
    sys.modules['antenv.axon_hooks'] = mod


# ---------------------------------------------------------------------------
# Host preprocessing: max-based straddled schedule + per-core slot data
# ---------------------------------------------------------------------------
def _host_prep(src, dst, et):
    src = np.asarray(src).astype(np.int64)
    dst = np.asarray(dst).astype(np.int64)
    et = np.asarray(et).astype(np.int64)

    seg = et * N + dst
    segcnt = np.bincount(seg, minlength=R * N)
    inv_seg = np.where(segcnt > 0, 1.0 / np.maximum(segcnt, 1),
                       0.0).astype(np.float32)

    core = dst // SHARD
    dloc = dst - core * SHARD
    tl = dloc // 128
    dl = (dloc % 128).astype(np.float32)
    gid = tl * R + et
    cls = (src >= SPLIT).astype(np.int64)

    bucket = (core * NG + gid) * 2 + cls
    cnts = np.bincount(bucket, minlength=NCORES * NG * 2).reshape(
        NCORES, NG, 2)
    slots_g = cnts.max(axis=0)                     # [NG, 2] uniform slots

    # class stream layout: groups in gid order, slots_g[g, cl] slots each
    slot0 = np.zeros((NG, 2), np.int64)
    slot0[:, 0] = np.cumsum(slots_g[:, 0]) - slots_g[:, 0]
    slot0[:, 1] = np.cumsum(slots_g[:, 1]) - slots_g[:, 1]
    S = [int(slots_g[:, 0].sum()), int(slots_g[:, 1].sum())]
    nchunks = [-(-S[0] // 128), -(-S[1] // 128)]
    NB = [max(1, -(-nchunks[0] // BATCH)), max(1, -(-nchunks[1] // BATCH))]

    # chunk segment tables: per class, per chunk, list of
    # (g, seg_idx, lo, hi) with slots [lo, hi) of the chunk (0-127 local).
    # A slot's mask target is dl' = seg_idx*128 + dst_in_tile.
    chunk_segs = [[], []]
    seg_of_slot = [np.zeros(max(S[0], 1), np.int16),
                   np.zeros(max(S[1], 1), np.int16)]
    for cl in (0, 1):
        g_iter = 0
        for k in range(nchunks[cl]):
            base = k * 128
            end = min(base + 128, S[cl])
            segs = []
            while g_iter < NG and slot0[g_iter, cl] + slots_g[g_iter, cl] \
                    <= base:
                g_iter += 1
            gi = g_iter
            while gi < NG and slot0[gi, cl] < end:
                lo = max(int(slot0[gi, cl]), base)
                hi = min(int(slot0[gi, cl] + slots_g[gi, cl]), end)
                if hi > lo:
                    si = len(segs)
                    segs.append((gi, si, lo - base, hi - base))
                    seg_of_slot[cl][lo:hi] = si
                gi += 1
            assert len(segs) <= MAXSEG, f"chunk spans {len(segs)} groups"
            chunk_segs[cl].append(segs)

    order = np.argsort(bucket, kind='stable')
    sk = bucket[order]
    nb = NCORES * NG * 2
    starts = np.searchsorted(sk, np.arange(nb))
    ends = np.searchsorted(sk, np.arange(nb) + 1)

    invv = inv_seg[seg]
    per_core = []
    for c in range(NCORES):
        idx_cls = [np.zeros(NB[0] * BATCH * 128, np.int32),
                   np.zeros(NB[1] * BATCH * 128, np.int32)]
        dl_arr = [np.full((128, nchunks[0]), -1.0, np.float32),
                  np.full((128, nchunks[1]), -1.0, np.float32)]
        inv_arr = [np.zeros((128, nchunks[0]), np.float32),
                   np.zeros((128, nchunks[1]), np.float32)]
        for cl in (0, 1):
            for g in range(NG):
                b = ((c * NG + g) * 2 + cl)
                e = order[starts[b]:ends[b]]
                n = len(e)
                if n == 0:
                    continue
                s0 = int(slot0[g, cl])
                idx_cls[cl][s0:s0 + n] = src[e] - (SPLIT if cl else 0)
                sl = s0 + np.arange(n)
                ks = sl // 128
                part = sl % 128
                segi = seg_of_slot[cl][sl].astype(np.float32)
                dl_arr[cl][part, ks] = segi * 128 + dl[e]
                inv_arr[cl][part, ks] = invv[e]
        wrapped = []
        for cl in (0, 1):
            a = idx_cls[cl]
            w = a.reshape(-1, 16).T.astype(np.int16)
            wrapped.append(np.ascontiguousarray(np.tile(w, (8, 1))))
        per_core.append(dict(idx_lo=wrapped[0], idx_hi=wrapped[1],
                             dl_lo=dl_arr[0], dl_hi=dl_arr[1],
                             inv_lo=inv_arr[0], inv_hi=inv_arr[1]))

    sched = dict(slots_g=slots_g, slot0=slot0, S=S, nchunks=nchunks, NB=NB,
                 chunk_segs=chunk_segs)
    return sched, per_core


def _pack_weights(W, nchunk, H):
    Rr = W.shape[0]
    out = np.zeros((128, Rr * nchunk * H), bf16)
    for r in range(Rr):
        for c in range(nchunk):
            out[:, (r * nchunk + c) * H:(r * nchunk + c + 1) * H] = \
                W[r, c * 128:(c + 1) * 128, :].astype(bf16)
    return out


def _pack_single(Wm, nchunk, H):
    out = np.zeros((128, nchunk * H), bf16)
    for c in range(nchunk):
        out[:, c * H:(c + 1) * H] = Wm[c * 128:(c + 1) * 128, :].astype(bf16)
    return out


def _make_xT(xf, c, width):
    FC = width // 128
    out = np.zeros((128, TILES * FC * 128), bf16)
    base = c * SHARD
    blk = np.zeros((width, TILES * 128), np.float32)
    blk[:, :SHARD] = xf[base:base + SHARD].T
    for t in range(TILES):
        for fc in range(FC):
            out[:, (t * FC + fc) * 128:(t * FC + fc + 1) * 128] = \
                blk[fc * 128:(fc + 1) * 128,
                    t * 128:(t + 1) * 128].astype(bf16)
    return out


# ---------------------------------------------------------------------------
# Device kernel builder (one launch per layer)
# ---------------------------------------------------------------------------
def _build_layer(layer, sched):
    import concourse.bass as bass
    import concourse.mybir as mybir
    from concourse.tile import TileContext
    from concourse.library_config import mlp
    from concourse.library_overlay import lower_extended_insts

    F = IN if layer == 1 else HID
    H = HID if layer == 1 else OUT
    FC = F // 128
    slots_g = sched['slots_g']
    nchunks = sched['nchunks']
    NB = sched['NB']
    chunk_segs = sched['chunk_segs']
    DLCOLS = [max(1, nchunks[0]), max(1, nchunks[1])]

    # ---- plan pass: drive order, first/last MM per group ----
    # events: ('chunk', cl, k) and ('close', g) in emission order
    events = []
    ptr = [0, 0]
    for t in range(TILES):
        for r in range(R):
            g = t * R + r
            for cl in (0, 1):
                while ptr[cl] < nchunks[cl] and \
                        chunk_segs[cl][ptr[cl]] and \
                        chunk_segs[cl][ptr[cl]][0][0] <= g:
                    events.append(('chunk', cl, ptr[cl]))
                    ptr[cl] += 1
            events.append(('close', g))
        events.append(('tile_end', t))
    assert ptr[0] == nchunks[0] and ptr[1] == nchunks[1], \
        f"unconsumed chunks {ptr} vs {nchunks}"
    # first/last (event_index, seg) per group + max simultaneous open psums
    touches = {}
    for ei, ev in enumerate(events):
        if ev[0] == 'chunk':
            cl, k = ev[1], ev[2]
            for (g, si, lo, hi) in chunk_segs[cl][k]:
                touches.setdefault(g, []).append((ei, si))
    first_touch = {g: v[0] for g, v in touches.items()}
    last_touch = {g: v[-1] for g, v in touches.items()}
    open_set = set()
    max_open = 0
    for ei, ev in enumerate(events):
        if ev[0] == 'chunk':
            cl, k = ev[1], ev[2]
            for (g, si, lo, hi) in chunk_segs[cl][k]:
                open_set.add(g)
                max_open = max(max_open, len(open_set))
        elif ev[0] == 'close':
            open_set.discard(ev[1])
    assert max_open <= 4, f"too many simultaneously open groups: {max_open}"

    # batch issuance: first event index that consumes each batch
    first_ev_of_batch = [[], []]
    for cl in (0, 1):
        seen = {}
        for ei, ev in enumerate(events):
            if ev[0] == 'chunk' and ev[1] == cl:
                b = ev[2] // BATCH
                if b not in seen:
                    seen[b] = ei
        first_ev_of_batch[cl] = [seen.get(b, 0) for b in range(NB[cl])]

    nc = bass.Bass()
    dt = mybir.dt
    xsrc = nc.dram_tensor('xsrc', [N, F], dt.bfloat16, kind='ExternalInput')
    idx_lo = nc.dram_tensor('idx_lo', [128, NB[0] * BATCH * 8], dt.int16,
                            kind='ExternalInput')
    idx_hi = nc.dram_tensor('idx_hi', [128, NB[1] * BATCH * 8], dt.int16,
                            kind='ExternalInput')
    dl_lo = nc.dram_tensor('dl_lo', [128, DLCOLS[0]], dt.float32,
                           kind='ExternalInput')
    dl_hi = nc.dram_tensor('dl_hi', [128, DLCOLS[1]], dt.float32,
                           kind='ExternalInput')
    inv_lo = nc.dram_tensor('inv_lo', [128, DLCOLS[0]], dt.float32,
                            kind='ExternalInput')
    inv_hi = nc.dram_tensor('inv_hi', [128, DLCOLS[1]], dt.float32,
                            kind='ExternalInput')
    iota = nc.dram_tensor('iota', [128, MAXSEG * 128], dt.float32,
                          kind='ExternalInput')
    Wsb = nc.dram_tensor('Wsb', [128, R * FC * H], dt.bfloat16,
                         kind='ExternalInput')
    rootsb = nc.dram_tensor('rootsb', [128, FC * H], dt.bfloat16,
                            kind='ExternalInput')
    brow = nc.dram_tensor('brow', [1, H], dt.bfloat16, kind='ExternalInput')
    xT = nc.dram_tensor('xT', [128, TILES * FC * 128], dt.bfloat16,
                        kind='ExternalInput')
    out_dt = dt.bfloat16 if layer == 1 else dt.float32
    yout = nc.dram_tensor('yout', [TILES * 128, H], out_dt,
                          kind='ExternalOutput')

    is_equal = mybir.AluOpType.is_equal
    mult = mybir.AluOpType.mult

    with TileContext(nc) as tc:
        with tc.tile_pool(name='const', bufs=1) as cp, \
             tc.tile_pool(name='glo', bufs=3) as glo, \
             tc.tile_pool(name='ghi', bufs=3) as ghi, \
             tc.tile_pool(name='xtp', bufs=2) as xtp, \
             tc.tile_pool(name='masks', bufs=6) as mp, \
             tc.tile_pool(name='asb', bufs=4) as asb, \
             tc.tile_pool(name='hout', bufs=3) as hp, \
             tc.tile_pool(name='pagg', bufs=5, space='PSUM') as pagg, \
             tc.tile_pool(name='pout', bufs=2, space='PSUM') as pout:

            nc.gpsimd.load_library(mlp)

            il_sb = cp.tile([128, NB[0] * BATCH * 8], dt.int16)
            nc.sync.dma_start(out=il_sb[:], in_=idx_lo[:])
            ih_sb = cp.tile([128, NB[1] * BATCH * 8], dt.int16)
            nc.sync.dma_start(out=ih_sb[:], in_=idx_hi[:])
            dll_sb = cp.tile([128, DLCOLS[0]], dt.float32)
            nc.sync.dma_start(out=dll_sb[:], in_=dl_lo[:])
            dlh_sb = cp.tile([128, DLCOLS[1]], dt.float32)
            nc.sync.dma_start(out=dlh_sb[:], in_=dl_hi[:])
            invl_sb = cp.tile([128, DLCOLS[0]], dt.float32)
            nc.sync.dma_start(out=invl_sb[:], in_=inv_lo[:])
            invh_sb = cp.tile([128, DLCOLS[1]], dt.float32)
            nc.sync.dma_start(out=invh_sb[:], in_=inv_hi[:])
            iota_sb = cp.tile([128, MAXSEG * 128], dt.float32)
            nc.sync.dma_start(out=iota_sb[:], in_=iota[:])
            W_sb = cp.tile([128, R * FC * H], dt.bfloat16)
            nc.sync.dma_start(out=W_sb[:], in_=Wsb[:])
            root_sb = cp.tile([128, FC * H], dt.bfloat16)
            nc.sync.dma_start(out=root_sb[:], in_=rootsb[:])
            b_sb = cp.tile([1, H], dt.bfloat16)
            nc.sync.dma_start(out=b_sb[:], in_=brow[:])
            ones_sb = cp.tile([1, 128], dt.bfloat16)
            nc.vector.memset(ones_sb[:], 1.0)

            idx_sbs = [il_sb, ih_sb]
            dl_sbs = [dll_sb, dlh_sb]
            inv_sbs = [invl_sb, invh_sb]
            srcs = [xsrc[:, :], xsrc[SPLIT:, :]]
            gpools = [glo, ghi]
            gtiles = [[None] * NB[0], [None] * NB[1]]
            next_b = [0, 0]
            nidx_reg = nc.gpsimd.to_reg(BATCH * 128)

            open_psum = {}
            copy_rr = [0]
            opsum_ref = [None]

            def issue_up_to(ei):
                for cl in (0, 1):
                    while (next_b[cl] < NB[cl]
                           and first_ev_of_batch[cl][next_b[cl]]
                           <= ei + 64):
                        k = next_b[cl]
                        gt = gpools[cl].tile([128, BATCH, F], dt.bfloat16,
                                             tag=f'g{cl}')
                        nc.gpsimd.dma_gather(
                            gt[:, :, :], srcs[cl],
                            idx_sbs[cl][:, k * (BATCH * 8):
                                        (k + 1) * (BATCH * 8)],
                            BATCH * 128, nidx_reg, F,
                            single_packet=False)
                        gtiles[cl][k] = gt
                        next_b[cl] += 1

            def emit_chunk(ei, cl, k):
                segs = chunk_segs[cl][k]
                if not segs:
                    return
                nseg = len(segs)
                bt = gtiles[cl][k // BATCH]
                col = k % BATCH
                m = mp.tile([128, nseg * 128], dt.bfloat16, tag='m')
                nc.vector.tensor_scalar(
                    out=m[:], in0=iota_sb[:, :nseg * 128],
                    scalar1=dl_sbs[cl][:, k:k + 1],
                    scalar2=inv_sbs[cl][:, k:k + 1],
                    op0=is_equal, op1=mult)
                for (g, si, lo, hi) in segs:
                    ap = open_psum.get(g)
                    if ap is None:
                        ap = pagg.tile([128, 512], dt.float32, tag='a')
                        open_psum[g] = ap
                    first = (first_touch[g] == (ei, si))
                    last = (last_touch[g] == (ei, si))
                    for fc in range(FC):
                        nc.tensor.matmul(
                            out=ap[:, fc * 128:(fc + 1) * 128],
                            lhsT=bt[:, col, fc * 128:(fc + 1) * 128],
                            rhs=m[:, si * 128:(si + 1) * 128],
                            start=(first and fc == 0),
                            stop=(last and fc == FC - 1))

            ei = 0
            for ev in events:
                if ev[0] == 'chunk':
                    issue_up_to(ei)
                    emit_chunk(ei, ev[1], ev[2])
                elif ev[0] == 'close':
                    g = ev[1]
                    t, r = g // R, g % R
                    if r == 0:
                        # open opsum for tile t, root + nothing yet
                        opsum = pout.tile([128, 512], dt.float32, tag='o')
                        opsum_ref[0] = opsum
                        xT_t = xtp.tile([128, FC * 128], dt.bfloat16,
                                        tag='xT')
                        nc.sync.dma_start(
                            out=xT_t[:],
                            in_=xT[:, t * FC * 128:(t + 1) * FC * 128])
                        for fc in range(FC):
                            nc.tensor.matmul(
                                out=opsum[:, :H],
                                lhsT=xT_t[:, fc * 128:(fc + 1) * 128],
                                rhs=root_sb[:, fc * H:(fc + 1) * H],
                                start=(fc == 0), stop=False)
                    opsum = opsum_ref[0]
                    ap = open_psum.pop(g, None)
                    if ap is not None:
                        aggT = asb.tile([128, FC * 128], dt.bfloat16,
                                        tag='at')
                        if copy_rr[0] % 4 == 3:
                            nc.vector.tensor_copy(out=aggT[:],
                                                  in_=ap[:, :FC * 128])
                        else:
                            nc.scalar.activation(
                                out=aggT[:], in_=ap[:, :FC * 128],
                                func=mybir.ActivationFunctionType.Copy)
                        copy_rr[0] += 1
                        for fc in range(FC):
                            nc.tensor.matmul(
                                out=opsum[:, :H],
                                lhsT=aggT[:, fc * 128:(fc + 1) * 128],
                                rhs=W_sb[:, (r * FC + fc) * H:
                                         (r * FC + fc + 1) * H],
                                start=False, stop=False)
                elif ev[0] == 'tile_end':
                    t = ev[1]
                    opsum = opsum_ref[0]
                    nc.tensor.matmul(
                        out=opsum[:, :H], lhsT=ones_sb[:], rhs=b_sb[:],
                        start=False, stop=True)
                    if layer == 1:
                        h_t = hp.tile([128, H], dt.bfloat16, tag='ht')
                        nc.scalar.activation(
                            out=h_t[:], in_=opsum[:, :H],
                            func=mybir.ActivationFunctionType.Relu)
                        nc.sync.dma_start(
                            out=yout[t * 128:(t + 1) * 128, :], in_=h_t[:])
                    else:
                        nrm2 = hp.tile([128, 1], dt.float32, tag='n2')
                        sq = hp.tile([128, OUT], dt.float32, tag='sq')
                        nc.scalar.activation(
                            out=sq[:], in_=opsum[:, :H],
                            func=mybir.ActivationFunctionType.Square,
                            accum_out=nrm2[:])
                        srt = hp.tile([128, 1], dt.float32, tag='srt')
                        nc.scalar.activation(
                            out=srt[:], in_=nrm2[:],
                            func=mybir.ActivationFunctionType.Sqrt)
                        nc.vector.tensor_scalar_max(srt[:], srt[:], 1e-12)
                        rcp = hp.tile([128, 1], dt.float32, tag='rcp')
                        nc.vector.reciprocal(rcp[:], srt[:])
                        o_t = hp.tile([128, OUT], dt.float32, tag='ot')
                        nc.scalar.activation(
                            out=o_t[:], in_=opsum[:, :H],
                            func=mybir.ActivationFunctionType.Copy,
                            scale=rcp[:])
                        nc.sync.dma_start(
                            out=yout[t * 128:(t + 1) * 128, :], in_=o_t[:])
                ei += 1
    lower_extended_insts(nc)
    return nc


def _run(nc, in_maps, trace=False):
    from concourse import bass_utils
    res = bass_utils.run_bass_kernel_spmd(
        nc, in_maps, core_ids=list(range(NCORES)), trace=trace)
    if trace:
        _last_traced[0] = res
    return res


# ---------------------------------------------------------------------------
# Entry point
# ---------------------------------------------------------------------------
def kernel(x, W1, root1, b1, W2, root2, b2, src, dst, edge_type,
           _trace=None):
    _install_tilefix()
    _install_ntff_hook()

    x = np.asarray(x, np.float32)
    sched, per_core = _host_prep(src, dst, edge_type)

    # iota value at column j equals j, compared against dl' = si*128 + dl;
    # fp32 so integers above 256 stay exact.
    iota_np = np.ascontiguousarray(np.broadcast_to(
        np.arange(MAXSEG * 128, dtype=np.float32), (128, MAXSEG * 128)))
    x_bf = np.ascontiguousarray(x.astype(bf16))
    W1p = _pack_weights(np.asarray(W1, np.float32), IN // 128, HID)
    r1p = _pack_single(np.asarray(root1, np.float32), IN // 128, HID)
    b1p = np.asarray(b1, np.float32)[None, :].astype(bf16)
    W2p = _pack_weights(np.asarray(W2, np.float32), HID // 128, OUT)
    r2p = _pack_single(np.asarray(root2, np.float32), HID // 128, OUT)
    b2p = np.asarray(b2, np.float32)[None, :].astype(bf16)

    def maps_for(c, Wp, rp, bp, xtab, xTc):
        pc = per_core[c]
        return dict(
            xsrc=xtab, idx_lo=pc['idx_lo'], idx_hi=pc['idx_hi'],
            dl_lo=pc['dl_lo'], dl_hi=pc['dl_hi'],
            inv_lo=pc['inv_lo'], inv_hi=pc['inv_hi'],
            iota=iota_np, Wsb=Wp, rootsb=rp, brow=bp, xT=xTc)

    # ---- layer 1 ----
    nc1 = _build_layer(1, sched)
    in_maps1 = [maps_for(c, W1p, r1p, b1p, x_bf, _make_xT(x, c, IN))
                for c in range(NCORES)]
    res1 = _run(nc1, in_maps1, trace=(_trace in ('l1', 'l1_0')))
    _pending_trace['l1'] = res1.exec_time_ns

    h_tab = np.empty((N, HID), bf16)
    for c in range(NCORES):
        h_tab[c * SHARD:(c + 1) * SHARD] = res1.results[c]['yout'][:SHARD]
    h_f32 = h_tab.astype(np.float32)

    # ---- layer 2 ----
    nc2 = _build_layer(2, sched)
    in_maps2 = [maps_for(c, W2p, r2p, b2p, h_tab, _make_xT(h_f32, c, HID))
                for c in range(NCORES)]
    res2 = _run(nc2, in_maps2, trace=(_trace in ('l2', 'l2_0')))
    _pending_trace['l2'] = res2.exec_time_ns

    out = np.empty((N, OUT), np.float32)
    for c in range(NCORES):
        out[c * SHARD:(c + 1) * SHARD] = \
            res2.results[c]['yout'][:SHARD].astype(np.float32)
    return out


# revision 20
# speedup vs baseline: 1.8421x; 1.1625x over previous
"""Trainium2 Bass kernel for 2-layer RGCN (mean aggregation) on 8 NeuronCores.

v3 design:
  - dst-sharded: core k owns destination rows [k*6250, (k+1)*6250) = 49 tiles
    of 128. Each core computes its output rows entirely; no collectives.
  - Gathers use the batched ant dma_gather (Q7 ucode). The HW bound is
    ~9.5ns per gathered row (SDMA per-descriptor pipeline), so the schedule
    minimizes gathered slots: per (dst_tile, relation, class) the slot count
    is the max edge count over the 8 cores (one SPMD program fits all), and
    slots pack densely into 128-row chunks that may straddle group
    boundaries (each straddled chunk does one extra masked matmul per
    feature chunk instead of padding the gather).
  - int16 gather indices: sources split into two classes by row range
    (src < 32768 from the table base, src >= 32768 from a +32768 offset).
  - Aggregation produces aggT directly: gathered messages are the matmul
    stationary operand, the one-hot dst mask (DVE-built, 1/cnt folded in)
    is the moving operand, accumulating agg^T[f, dst] per group in PSUM.
  - Transform per (tile, relation): aggT chunks x W_r -> opsum [dst, H]
    in PSUM along with x@root and bias.
  - One launch per layer; layer 1 output returns to the host, which
    assembles the h table (pure data movement) for layer 2's gathers.
  - bf16 operands, fp32 PSUM accumulation (~0.3% rel error).
"""
import numpy as np
import ml_dtypes

N = 50000
E = 800000
R = 8
IN, HID, OUT = 512, 256, 512
NCORES = 8
SHARD = 6250
TILES = 49                 # ceil(6250/128)
NG = TILES * R             # groups per core; gid = t*R + r
SPLIT = 32768              # int16-safe gather index split
BATCH = 8                  # chunks per dma_gather call
MAXSEG = 8                 # max group-segments per chunk (iota width)
bf16 = ml_dtypes.bfloat16

_pending_trace = {"l1": None, "l2": None}
_last_traced = [None]


# ---------------------------------------------------------------------------
# Workarounds for this container's walrus build (single sync-wait per
# instruction) and missing NTFF profile hook under axon.
# ---------------------------------------------------------------------------
def _install_tilefix():
    import concourse.mybir as mybir
    import concourse.tile as tile_mod
    from concourse.vector_clock import ScopedClock

    if getattr(tile_mod.TileContext, "_rgcn_patched", False):
        return
    counter = [0]

    def split_multiwaits(nc):
        for f in nc.m.functions:
            for bb in f.blocks:
                out = []
                changed = False
                for inst in bb.instructions:
                    si = inst.sync_info
                    waits = list(si.on_wait) if si is not None else []
                    if len(waits) > 1:
                        changed = True
                        for w in waits[:-1]:
                            counter[0] += 1
                            nop = mybir.InstNoOp(
                                name=f"I-wsplit-{counter[0]}", ins=[], outs=[])
                            nop.engine = inst.engine
                            nop.sync_info = mybir.SyncInfo(
                                on_wait=[w], on_update=[])
                            nc.register_instruction(nop, overwrite=True)
                            out.append(nop)
                        si.on_wait = waits[-1:]
                    out.append(inst)
                if changed:
                    bb.instructions = out

    def patched_drain_and_barrier(self, tick_clock, wait_clock):
        nc = self.nc
        drain_inst = nc.sync.drain()
        wait_clock.add_sem_waits(
            drain_inst.ins, ScopedClock({None: tick_clock.global_clock}))
        nc.all_engine_barrier()
        assert self.sems is not None
        popped = nc._tile_sem_poison_stack.pop()
        assert popped is self._sem_poison
        nc.clear_and_free_semaphores(list(self.sems.allocated().values()))
        nc.all_engine_barrier()
        split_multiwaits(nc)

    tile_mod.TileContext._drain_and_barrier = patched_drain_and_barrier
    tile_mod.TileContext._rgcn_patched = True


def _install_ntff_hook():
    import sys, types
    if 'antenv.axon_hooks' in sys.modules:
        return
    try:
        try:
            from trn_agent_boot.trn_boot import _ntff_profile_via_ctypes
        except ImportError:
            sys.path.insert(0, '/root/.axon_site')
            from trn_agent_boot.trn_boot import _ntff_profile_via_ctypes
        hook = _ntff_profile_via_ctypes('/opt/axon/libaxon_pjrt.so')
    except Exception:
        return
    mod = types.ModuleType('antenv.axon_hooks')
    mod.get_axon_ntff_profile_hook = lambda: hook
    mod.set_axon_ntff_profile_hook = lambda h: None
    sys.modules['antenv.axon_hooks'] = mod


# ---------------------------------------------------------------------------
# Host preprocessing: max-based straddled schedule + per-core slot data
# ---------------------------------------------------------------------------
def _host_prep(src, dst, et):
    src = np.asarray(src).astype(np.int64)
    dst = np.asarray(dst).astype(np.int64)
    et = np.asarray(et).astype(np.int64)

    seg = et * N + dst
    segcnt = np.bincount(seg, minlength=R * N)
    inv_seg = np.where(segcnt > 0, 1.0 / np.maximum(segcnt, 1),
                       0.0).astype(np.float32)

    core = dst // SHARD
    dloc = dst - core * SHARD
    tl = dloc // 128
    dl = (dloc % 128).astype(np.float32)
    gid = tl * R + et
    cls = (src >= SPLIT).astype(np.int64)

    bucket = (core * NG + gid) * 2 + cls
    cnts = np.bincount(bucket, minlength=NCORES * NG * 2).reshape(
        NCORES, NG, 2)
    slots_g = cnts.max(axis=0)                     # [NG, 2] uniform slots

    # class stream layout: groups in gid order, slots_g[g, cl] slots each
    slot0 = np.zeros((NG, 2), np.int64)
    slot0[:, 0] = np.cumsum(slots_g[:, 0]) - slots_g[:, 0]
    slot0[:, 1] = np.cumsum(slots_g[:, 1]) - slots_g[:, 1]
    S = [int(slots_g[:, 0].sum()), int(slots_g[:, 1].sum())]
    nchunks = [-(-S[0] // 128), -(-S[1] // 128)]
    NB = [max(1, -(-nchunks[0] // BATCH)), max(1, -(-nchunks[1] // BATCH))]

    # chunk segment tables: per class, per chunk, list of
    # (g, seg_idx, lo, hi) with slots [lo, hi) of the chunk (0-127 local).
    # A slot's mask target is dl' = seg_idx*128 + dst_in_tile.
    chunk_segs = [[], []]
    seg_of_slot = [np.zeros(max(S[0], 1), np.int16),
                   np.zeros(max(S[1], 1), np.int16)]
    for cl in (0, 1):
        g_iter = 0
        for k in range(nchunks[cl]):
            base = k * 128
            end = min(base + 128, S[cl])
            segs = []
            while g_iter < NG and slot0[g_iter, cl] + slots_g[g_iter, cl] \
                    <= base:
                g_iter += 1
            gi = g_iter
            while gi < NG and slot0[gi, cl] < end:
                lo = max(int(slot0[gi, cl]), base)
                hi = min(int(slot0[gi, cl] + slots_g[gi, cl]), end)
                if hi > lo:
                    si = len(segs)
                    segs.append((gi, si, lo - base, hi - base))
                    seg_of_slot[cl][lo:hi] = si
                gi += 1
            assert len(segs) <= MAXSEG, f"chunk spans {len(segs)} groups"
            chunk_segs[cl].append(segs)

    order = np.argsort(bucket, kind='stable')
    sk = bucket[order]
    nb = NCORES * NG * 2
    starts = np.searchsorted(sk, np.arange(nb))
    ends = np.searchsorted(sk, np.arange(nb) + 1)

    invv = inv_seg[seg]
    per_core = []
    for c in range(NCORES):
        idx_cls = [np.zeros(NB[0] * BATCH * 128, np.int32),
                   np.zeros(NB[1] * BATCH * 128, np.int32)]
        dl_arr = [np.full((128, nchunks[0]), -1.0, np.float32),
                  np.full((128, nchunks[1]), -1.0, np.float32)]
        inv_arr = [np.zeros((128, nchunks[0]), np.float32),
                   np.zeros((128, nchunks[1]), np.float32)]
        for cl in (0, 1):
            for g in range(NG):
                b = ((c * NG + g) * 2 + cl)
                e = order[starts[b]:ends[b]]
                n = len(e)
                if n == 0:
                    continue
                s0 = int(slot0[g, cl])
                idx_cls[cl][s0:s0 + n] = src[e] - (SPLIT if cl else 0)
                sl = s0 + np.arange(n)
                ks = sl // 128
                part = sl % 128
                segi = seg_of_slot[cl][sl].astype(np.float32)
                dl_arr[cl][part, ks] = segi * 128 + dl[e]
                inv_arr[cl][part, ks] = invv[e]
        wrapped = []
        for cl in (0, 1):
            a = idx_cls[cl]
            w = a.reshape(-1, 16).T.astype(np.int16)
            wrapped.append(np.ascontiguousarray(np.tile(w, (8, 1))))
        per_core.append(dict(idx_lo=wrapped[0], idx_hi=wrapped[1],
                             dl_lo=dl_arr[0], dl_hi=dl_arr[1],
                             inv_lo=inv_arr[0], inv_hi=inv_arr[1]))

    sched = dict(slots_g=slots_g, slot0=slot0, S=S, nchunks=nchunks, NB=NB,
                 chunk_segs=chunk_segs)
    return sched, per_core


def _pack_weights(W, nchunk, H):
    Rr = W.shape[0]
    out = np.zeros((128, Rr * nchunk * H), bf16)
    for r in range(Rr):
        for c in range(nchunk):
            out[:, (r * nchunk + c) * H:(r * nchunk + c + 1) * H] = \
                W[r, c * 128:(c + 1) * 128, :].astype(bf16)
    return out


def _pack_single(Wm, nchunk, H):
    out = np.zeros((128, nchunk * H), bf16)
    for c in range(nchunk):
        out[:, c * H:(c + 1) * H] = Wm[c * 128:(c + 1) * 128, :].astype(bf16)
    return out


def _make_xT(xf, c, width):
    FC = width // 128
    out = np.zeros((128, TILES * FC * 128), bf16)
    base = c * SHARD
    blk = np.zeros((width, TILES * 128), np.float32)
    blk[:, :SHARD] = xf[base:base + SHARD].T
    for t in range(TILES):
        for fc in range(FC):
            out[:, (t * FC + fc) * 128:(t * FC + fc + 1) * 128] = \
                blk[fc * 128:(fc + 1) * 128,
                    t * 128:(t + 1) * 128].astype(bf16)
    return out


# ---------------------------------------------------------------------------
# Device kernel builder (one launch per layer)
# ---------------------------------------------------------------------------
def _build_layer(layer, sched):
    import concourse.bass as bass
    import concourse.mybir as mybir
    from concourse.tile import TileContext
    from concourse.library_config import mlp
    from concourse.library_overlay import lower_extended_insts

    F = IN if layer == 1 else HID
    H = HID if layer == 1 else OUT
    FC = F // 128
    slots_g = sched['slots_g']
    nchunks = sched['nchunks']
    NB = sched['NB']
    chunk_segs = sched['chunk_segs']
    DLCOLS = [max(1, nchunks[0]), max(1, nchunks[1])]

    # ---- plan pass: drive order, first/last MM per group ----
    # events: ('chunk', cl, k) and ('close', g) in emission order
    events = []
    ptr = [0, 0]
    for t in range(TILES):
        for r in range(R):
            g = t * R + r
            for cl in (0, 1):
                while ptr[cl] < nchunks[cl] and \
                        chunk_segs[cl][ptr[cl]] and \
                        chunk_segs[cl][ptr[cl]][0][0] <= g:
                    events.append(('chunk', cl, ptr[cl]))
                    ptr[cl] += 1
            events.append(('close', g))
        events.append(('tile_end', t))
    assert ptr[0] == nchunks[0] and ptr[1] == nchunks[1], \
        f"unconsumed chunks {ptr} vs {nchunks}"
    # first/last (event_index, seg) per group + max simultaneous open psums
    touches = {}
    for ei, ev in enumerate(events):
        if ev[0] == 'chunk':
            cl, k = ev[1], ev[2]
            for (g, si, lo, hi) in chunk_segs[cl][k]:
                touches.setdefault(g, []).append((ei, si))
    first_touch = {g: v[0] for g, v in touches.items()}
    last_touch = {g: v[-1] for g, v in touches.items()}
    open_set = set()
    max_open = 0
    for ei, ev in enumerate(events):
        if ev[0] == 'chunk':
            cl, k = ev[1], ev[2]
            for (g, si, lo, hi) in chunk_segs[cl][k]:
                open_set.add(g)
                max_open = max(max_open, len(open_set))
        elif ev[0] == 'close':
            open_set.discard(ev[1])
    assert max_open <= 4, f"too many simultaneously open groups: {max_open}"

    # batch issuance: first event index that consumes each batch
    first_ev_of_batch = [[], []]
    for cl in (0, 1):
        seen = {}
        for ei, ev in enumerate(events):
            if ev[0] == 'chunk' and ev[1] == cl:
                b = ev[2] // BATCH
                if b not in seen:
                    seen[b] = ei
        first_ev_of_batch[cl] = [seen.get(b, 0) for b in range(NB[cl])]

    nc = bass.Bass()
    dt = mybir.dt
    xsrc = nc.dram_tensor('xsrc', [N, F], dt.bfloat16, kind='ExternalInput')
    idx_lo = nc.dram_tensor('idx_lo', [128, NB[0] * BATCH * 8], dt.int16,
                            kind='ExternalInput')
    idx_hi = nc.dram_tensor('idx_hi', [128, NB[1] * BATCH * 8], dt.int16,
                            kind='ExternalInput')
    dl_lo = nc.dram_tensor('dl_lo', [128, DLCOLS[0]], dt.float32,
                           kind='ExternalInput')
    dl_hi = nc.dram_tensor('dl_hi', [128, DLCOLS[1]], dt.float32,
                           kind='ExternalInput')
    inv_lo = nc.dram_tensor('inv_lo', [128, DLCOLS[0]], dt.float32,
                            kind='ExternalInput')
    inv_hi = nc.dram_tensor('inv_hi', [128, DLCOLS[1]], dt.float32,
                            kind='ExternalInput')
    iota = nc.dram_tensor('iota', [128, MAXSEG * 128], dt.float16,
                          kind='ExternalInput')
    Wsb = nc.dram_tensor('Wsb', [128, R * FC * H], dt.bfloat16,
                         kind='ExternalInput')
    rootsb = nc.dram_tensor('rootsb', [128, FC * H], dt.bfloat16,
                            kind='ExternalInput')
    brow = nc.dram_tensor('brow', [1, H], dt.bfloat16, kind='ExternalInput')
    xT = nc.dram_tensor('xT', [128, TILES * FC * 128], dt.bfloat16,
                        kind='ExternalInput')
    out_dt = dt.bfloat16 if layer == 1 else dt.float32
    yout = nc.dram_tensor('yout', [TILES * 128, H], out_dt,
                          kind='ExternalOutput')

    is_equal = mybir.AluOpType.is_equal
    mult = mybir.AluOpType.mult

    with TileContext(nc) as tc:
        with tc.tile_pool(name='const', bufs=1) as cp, \
             tc.tile_pool(name='glo', bufs=3) as glo, \
             tc.tile_pool(name='ghi', bufs=3) as ghi, \
             tc.tile_pool(name='xtp', bufs=2) as xtp, \
             tc.tile_pool(name='masks', bufs=6) as mp, \
             tc.tile_pool(name='asb', bufs=4) as asb, \
             tc.tile_pool(name='hout', bufs=3) as hp, \
             tc.tile_pool(name='pagg', bufs=5, space='PSUM') as pagg, \
             tc.tile_pool(name='pout', bufs=2, space='PSUM') as pout:

            nc.gpsimd.load_library(mlp)

            il_sb = cp.tile([128, NB[0] * BATCH * 8], dt.int16)
            nc.sync.dma_start(out=il_sb[:], in_=idx_lo[:])
            ih_sb = cp.tile([128, NB[1] * BATCH * 8], dt.int16)
            nc.sync.dma_start(out=ih_sb[:], in_=idx_hi[:])
            dll_sb = cp.tile([128, DLCOLS[0]], dt.float32)
            nc.sync.dma_start(out=dll_sb[:], in_=dl_lo[:])
            dlh_sb = cp.tile([128, DLCOLS[1]], dt.float32)
            nc.sync.dma_start(out=dlh_sb[:], in_=dl_hi[:])
            invl_sb = cp.tile([128, DLCOLS[0]], dt.float32)
            nc.sync.dma_start(out=invl_sb[:], in_=inv_lo[:])
            invh_sb = cp.tile([128, DLCOLS[1]], dt.float32)
            nc.sync.dma_start(out=invh_sb[:], in_=inv_hi[:])
            iota_sb = cp.tile([128, MAXSEG * 128], dt.float16)
            nc.sync.dma_start(out=iota_sb[:], in_=iota[:])
            W_sb = cp.tile([128, R * FC * H], dt.bfloat16)
            nc.sync.dma_start(out=W_sb[:], in_=Wsb[:])
            root_sb = cp.tile([128, FC * H], dt.bfloat16)
            nc.sync.dma_start(out=root_sb[:], in_=rootsb[:])
            b_sb = cp.tile([1, H], dt.bfloat16)
            nc.sync.dma_start(out=b_sb[:], in_=brow[:])
            ones_sb = cp.tile([1, 128], dt.bfloat16)
            nc.vector.memset(ones_sb[:], 1.0)

            idx_sbs = [il_sb, ih_sb]
            dl_sbs = [dll_sb, dlh_sb]
            inv_sbs = [invl_sb, invh_sb]
            srcs = [xsrc[:, :], xsrc[SPLIT:, :]]
            gpools = [glo, ghi]
            gtiles = [[None] * NB[0], [None] * NB[1]]
            next_b = [0, 0]
            nidx_reg = nc.gpsimd.to_reg(BATCH * 128)

            open_psum = {}
            copy_rr = [0]
            opsum_ref = [None]

            def issue_up_to(ei):
                for cl in (0, 1):
                    while (next_b[cl] < NB[cl]
                           and first_ev_of_batch[cl][next_b[cl]]
                           <= ei + 64):
                        k = next_b[cl]
                        gt = gpools[cl].tile([128, BATCH, F], dt.bfloat16,
                                             tag=f'g{cl}')
                        nc.gpsimd.dma_gather(
                            gt[:, :, :], srcs[cl],
                            idx_sbs[cl][:, k * (BATCH * 8):
                                        (k + 1) * (BATCH * 8)],
                            BATCH * 128, nidx_reg, F,
                            single_packet=False)
                        gtiles[cl][k] = gt
                        next_b[cl] += 1

            def emit_chunk(ei, cl, k):
                segs = chunk_segs[cl][k]
                if not segs:
                    return
                nseg = len(segs)
                bt = gtiles[cl][k // BATCH]
                col = k % BATCH
                m = mp.tile([128, nseg * 128], dt.bfloat16, tag='m')
                nc.vector.tensor_scalar(
                    out=m[:], in0=iota_sb[:, :nseg * 128],
                    scalar1=dl_sbs[cl][:, k:k + 1],
                    scalar2=inv_sbs[cl][:, k:k + 1],
                    op0=is_equal, op1=mult)
                for (g, si, lo, hi) in segs:
                    ap = open_psum.get(g)
                    if ap is None:
                        ap = pagg.tile([128, 512], dt.float32, tag='a')
                        open_psum[g] = ap
                    first = (first_touch[g] == (ei, si))
                    last = (last_touch[g] == (ei, si))
                    for fc in range(FC):
                        nc.tensor.matmul(
                            out=ap[:, fc * 128:(fc + 1) * 128],
                            lhsT=bt[:, col, fc * 128:(fc + 1) * 128],
                            rhs=m[:, si * 128:(si + 1) * 128],
                            start=(first and fc == 0),
                            stop=(last and fc == FC - 1))

            ei = 0
            for ev in events:
                if ev[0] == 'chunk':
                    issue_up_to(ei)
                    emit_chunk(ei, ev[1], ev[2])
                elif ev[0] == 'close':
                    g = ev[1]
                    t, r = g // R, g % R
                    if r == 0:
                        # open opsum for tile t, root + nothing yet
                        opsum = pout.tile([128, 512], dt.float32, tag='o')
                        opsum_ref[0] = opsum
                        xT_t = xtp.tile([128, FC * 128], dt.bfloat16,
                                        tag='xT')
                        nc.sync.dma_start(
                            out=xT_t[:],
                            in_=xT[:, t * FC * 128:(t + 1) * FC * 128])
                        for fc in range(FC):
                            nc.tensor.matmul(
                                out=opsum[:, :H],
                                lhsT=xT_t[:, fc * 128:(fc + 1) * 128],
                                rhs=root_sb[:, fc * H:(fc + 1) * H],
                                start=(fc == 0), stop=False)
                    opsum = opsum_ref[0]
                    ap = open_psum.pop(g, None)
                    if ap is not None:
                        aggT = asb.tile([128, FC * 128], dt.bfloat16,
                                        tag='at')
                        nc.scalar.activation(
                            out=aggT[:], in_=ap[:, :FC * 128],
                            func=mybir.ActivationFunctionType.Copy)
                        copy_rr[0] += 1
                        for fc in range(FC):
                            nc.tensor.matmul(
                                out=opsum[:, :H],
                                lhsT=aggT[:, fc * 128:(fc + 1) * 128],
                                rhs=W_sb[:, (r * FC + fc) * H:
                                         (r * FC + fc + 1) * H],
                                start=False, stop=False)
                elif ev[0] == 'tile_end':
                    t = ev[1]
                    opsum = opsum_ref[0]
                    nc.tensor.matmul(
                        out=opsum[:, :H], lhsT=ones_sb[:], rhs=b_sb[:],
                        start=False, stop=True)
                    if layer == 1:
                        h_t = hp.tile([128, H], dt.bfloat16, tag='ht')
                        nc.scalar.activation(
                            out=h_t[:], in_=opsum[:, :H],
                            func=mybir.ActivationFunctionType.Relu)
                        nc.sync.dma_start(
                            out=yout[t * 128:(t + 1) * 128, :], in_=h_t[:])
                    else:
                        nrm2 = hp.tile([128, 1], dt.float32, tag='n2')
                        sq = hp.tile([128, OUT], dt.float32, tag='sq')
                        nc.scalar.activation(
                            out=sq[:], in_=opsum[:, :H],
                            func=mybir.ActivationFunctionType.Square,
                            accum_out=nrm2[:])
                        srt = hp.tile([128, 1], dt.float32, tag='srt')
                        nc.scalar.activation(
                            out=srt[:], in_=nrm2[:],
                            func=mybir.ActivationFunctionType.Sqrt)
                        nc.vector.tensor_scalar_max(srt[:], srt[:], 1e-12)
                        rcp = hp.tile([128, 1], dt.float32, tag='rcp')
                        nc.vector.reciprocal(rcp[:], srt[:])
                        o_t = hp.tile([128, OUT], dt.float32, tag='ot')
                        nc.scalar.activation(
                            out=o_t[:], in_=opsum[:, :H],
                            func=mybir.ActivationFunctionType.Copy,
                            scale=rcp[:])
                        nc.sync.dma_start(
                            out=yout[t * 128:(t + 1) * 128, :], in_=o_t[:])
                ei += 1
    lower_extended_insts(nc)
    return nc


def _run(nc, in_maps, trace=False):
    from concourse import bass_utils
    res = bass_utils.run_bass_kernel_spmd(
        nc, in_maps, core_ids=list(range(NCORES)), trace=trace)
    if trace:
        _last_traced[0] = res
    return res


# ---------------------------------------------------------------------------
# Entry point
# ---------------------------------------------------------------------------
def kernel(x, W1, root1, b1, W2, root2, b2, src, dst, edge_type,
           _trace=None):
    _install_tilefix()
    _install_ntff_hook()

    x = np.asarray(x, np.float32)
    sched, per_core = _host_prep(src, dst, edge_type)

    # iota value at column j equals j, compared against dl' = si*128 + dl;
    # fp16 keeps integers up to 2048 exact at full (2-byte) DVE rate.
    iota_np = np.ascontiguousarray(np.broadcast_to(
        np.arange(MAXSEG * 128, dtype=np.float32),
        (128, MAXSEG * 128))).astype(np.float16)
    x_bf = np.ascontiguousarray(x.astype(bf16))
    W1p = _pack_weights(np.asarray(W1, np.float32), IN // 128, HID)
    r1p = _pack_single(np.asarray(root1, np.float32), IN // 128, HID)
    b1p = np.asarray(b1, np.float32)[None, :].astype(bf16)
    W2p = _pack_weights(np.asarray(W2, np.float32), HID // 128, OUT)
    r2p = _pack_single(np.asarray(root2, np.float32), HID // 128, OUT)
    b2p = np.asarray(b2, np.float32)[None, :].astype(bf16)

    def maps_for(c, Wp, rp, bp, xtab, xTc):
        pc = per_core[c]
        return dict(
            xsrc=xtab, idx_lo=pc['idx_lo'], idx_hi=pc['idx_hi'],
            dl_lo=pc['dl_lo'], dl_hi=pc['dl_hi'],
            inv_lo=pc['inv_lo'], inv_hi=pc['inv_hi'],
            iota=iota_np, Wsb=Wp, rootsb=rp, brow=bp, xT=xTc)

    # ---- layer 1 ----
    nc1 = _build_layer(1, sched)
    in_maps1 = [maps_for(c, W1p, r1p, b1p, x_bf, _make_xT(x, c, IN))
                for c in range(NCORES)]
    res1 = _run(nc1, in_maps1, trace=(_trace in ('l1', 'l1_0')))
    _pending_trace['l1'] = res1.exec_time_ns

    h_tab = np.empty((N, HID), bf16)
    for c in range(NCORES):
        h_tab[c * SHARD:(c + 1) * SHARD] = res1.results[c]['yout'][:SHARD]
    h_f32 = h_tab.astype(np.float32)

    # ---- layer 2 ----
    nc2 = _build_layer(2, sched)
    in_maps2 = [maps_for(c, W2p, r2p, b2p, h_tab, _make_xT(h_f32, c, HID))
                for c in range(NCORES)]
    res2 = _run(nc2, in_maps2, trace=(_trace in ('l2', 'l2_0')))
    _pending_trace['l2'] = res2.exec_time_ns

    out = np.empty((N, OUT), np.float32)
    for c in range(NCORES):
        out[c * SHARD:(c + 1) * SHARD] = \
            res2.results[c]['yout'][:SHARD].astype(np.float32)
    return out


# revision 22
# speedup vs baseline: 1.9274x; 1.0463x over previous
"""Trainium2 Bass kernel for 2-layer RGCN (mean aggregation) on 8 NeuronCores.

v3 design:
  - dst-sharded: core k owns destination rows [k*6250, (k+1)*6250) = 49 tiles
    of 128. Each core computes its output rows entirely; no collectives.
  - Gathers use the batched ant dma_gather (Q7 ucode). The HW bound is
    ~9.5ns per gathered row (SDMA per-descriptor pipeline), so the schedule
    minimizes gathered slots: per (dst_tile, relation, class) the slot count
    is the max edge count over the 8 cores (one SPMD program fits all), and
    slots pack densely into 128-row chunks that may straddle group
    boundaries (each straddled chunk does one extra masked matmul per
    feature chunk instead of padding the gather).
  - int16 gather indices: sources split into two classes by row range
    (src < 32768 from the table base, src >= 32768 from a +32768 offset).
  - Aggregation produces aggT directly: gathered messages are the matmul
    stationary operand, the one-hot dst mask (DVE-built, 1/cnt folded in)
    is the moving operand, accumulating agg^T[f, dst] per group in PSUM.
  - Transform per (tile, relation): aggT chunks x W_r -> opsum [dst, H]
    in PSUM along with x@root and bias.
  - One launch per layer; layer 1 output returns to the host, which
    assembles the h table (pure data movement) for layer 2's gathers.
  - bf16 operands, fp32 PSUM accumulation (~0.3% rel error).
"""
import numpy as np
import ml_dtypes

N = 50000
E = 800000
R = 8
IN, HID, OUT = 512, 256, 512
NCORES = 8
SHARD = 6250
TILES = 49                 # ceil(6250/128)
NG = TILES * R             # groups per core; gid = t*R + r
SPLIT = 32768              # int16-safe gather index split
BATCH = 8                  # chunks per dma_gather call
MAXSEG = 8                 # max group-segments per chunk (iota width)
bf16 = ml_dtypes.bfloat16

_pending_trace = {"l1": None, "l2": None}
_last_traced = [None]


# ---------------------------------------------------------------------------
# Workarounds for this container's walrus build (single sync-wait per
# instruction) and missing NTFF profile hook under axon.
# ---------------------------------------------------------------------------
def _install_tilefix():
    import concourse.mybir as mybir
    import concourse.tile as tile_mod
    from concourse.vector_clock import ScopedClock

    if getattr(tile_mod.TileContext, "_rgcn_patched", False):
        return
    counter = [0]

    def split_multiwaits(nc):
        for f in nc.m.functions:
            for bb in f.blocks:
                out = []
                changed = False
                for inst in bb.instructions:
                    si = inst.sync_info
                    waits = list(si.on_wait) if si is not None else []
                    if len(waits) > 1:
                        changed = True
                        for w in waits[:-1]:
                            counter[0] += 1
                            nop = mybir.InstNoOp(
                                name=f"I-wsplit-{counter[0]}", ins=[], outs=[])
                            nop.engine = inst.engine
                            nop.sync_info = mybir.SyncInfo(
                                on_wait=[w], on_update=[])
                            nc.register_instruction(nop, overwrite=True)
                            out.append(nop)
                        si.on_wait = waits[-1:]
                    out.append(inst)
                if changed:
                    bb.instructions = out

    def patched_drain_and_barrier(self, tick_clock, wait_clock):
        nc = self.nc
        drain_inst = nc.sync.drain()
        wait_clock.add_sem_waits(
            drain_inst.ins, ScopedClock({None: tick_clock.global_clock}))
        nc.all_engine_barrier()
        assert self.sems is not None
        popped = nc._tile_sem_poison_stack.pop()
        assert popped is self._sem_poison
        nc.clear_and_free_semaphores(list(self.sems.allocated().values()))
        nc.all_engine_barrier()
        split_multiwaits(nc)

    tile_mod.TileContext._drain_and_barrier = patched_drain_and_barrier
    tile_mod.TileContext._rgcn_patched = True


def _install_ntff_hook():
    import sys, types
    if 'antenv.axon_hooks' in sys.modules:
        return
    try:
        try:
            from trn_agent_boot.trn_boot import _ntff_profile_via_ctypes
        except ImportError:
            sys.path.insert(0, '/root/.axon_site')
            from trn_agent_boot.trn_boot import _ntff_profile_via_ctypes
        hook = _ntff_profile_via_ctypes('/opt/axon/libaxon_pjrt.so')
    except Exception:
        return
    mod = types.ModuleType('antenv.axon_hooks')
    mod.get_axon_ntff_profile_hook = lambda: hook
    mod.set_axon_ntff_profile_hook = lambda h: None
    sys.modules['antenv.axon_hooks'] = mod


# ---------------------------------------------------------------------------
# Host preprocessing: max-based straddled schedule + per-core slot data
# ---------------------------------------------------------------------------
def _host_prep(src, dst, et):
    src = np.asarray(src).astype(np.int64)
    dst = np.asarray(dst).astype(np.int64)
    et = np.asarray(et).astype(np.int64)

    seg = et * N + dst
    segcnt = np.bincount(seg, minlength=R * N)
    inv_seg = np.where(segcnt > 0, 1.0 / np.maximum(segcnt, 1),
                       0.0).astype(np.float32)

    core = dst // SHARD
    dloc = dst - core * SHARD
    tl = dloc // 128
    dl = (dloc % 128).astype(np.float32)
    gid = tl * R + et
    cls = (src >= SPLIT).astype(np.int64)

    bucket = (core * NG + gid) * 2 + cls
    cnts = np.bincount(bucket, minlength=NCORES * NG * 2).reshape(
        NCORES, NG, 2)
    slots_g = cnts.max(axis=0)                     # [NG, 2] uniform slots

    # class stream layout: groups in gid order, slots_g[g, cl] slots each
    slot0 = np.zeros((NG, 2), np.int64)
    slot0[:, 0] = np.cumsum(slots_g[:, 0]) - slots_g[:, 0]
    slot0[:, 1] = np.cumsum(slots_g[:, 1]) - slots_g[:, 1]
    S = [int(slots_g[:, 0].sum()), int(slots_g[:, 1].sum())]
    nchunks = [-(-S[0] // 128), -(-S[1] // 128)]
    NB = [max(1, -(-nchunks[0] // BATCH)), max(1, -(-nchunks[1] // BATCH))]

    # chunk segment tables: per class, per chunk, list of
    # (g, seg_idx, lo, hi) with slots [lo, hi) of the chunk (0-127 local).
    # A slot's mask target is dl' = seg_idx*128 + dst_in_tile.
    chunk_segs = [[], []]
    seg_of_slot = [np.zeros(max(S[0], 1), np.int16),
                   np.zeros(max(S[1], 1), np.int16)]
    for cl in (0, 1):
        g_iter = 0
        for k in range(nchunks[cl]):
            base = k * 128
            end = min(base + 128, S[cl])
            segs = []
            while g_iter < NG and slot0[g_iter, cl] + slots_g[g_iter, cl] \
                    <= base:
                g_iter += 1
            gi = g_iter
            while gi < NG and slot0[gi, cl] < end:
                lo = max(int(slot0[gi, cl]), base)
                hi = min(int(slot0[gi, cl] + slots_g[gi, cl]), end)
                if hi > lo:
                    si = len(segs)
                    segs.append((gi, si, lo - base, hi - base))
                    seg_of_slot[cl][lo:hi] = si
                gi += 1
            assert len(segs) <= MAXSEG, f"chunk spans {len(segs)} groups"
            chunk_segs[cl].append(segs)

    order = np.argsort(bucket, kind='stable')
    sk = bucket[order]
    nb = NCORES * NG * 2
    starts = np.searchsorted(sk, np.arange(nb))
    ends = np.searchsorted(sk, np.arange(nb) + 1)

    invv = inv_seg[seg]
    per_core = []
    for c in range(NCORES):
        idx_cls = [np.zeros(NB[0] * BATCH * 128, np.int32),
                   np.zeros(NB[1] * BATCH * 128, np.int32)]
        dl_arr = [np.full((128, nchunks[0]), -1.0, np.float32),
                  np.full((128, nchunks[1]), -1.0, np.float32)]
        inv_arr = [np.zeros((128, nchunks[0]), np.float32),
                   np.zeros((128, nchunks[1]), np.float32)]
        for cl in (0, 1):
            for g in range(NG):
                b = ((c * NG + g) * 2 + cl)
                e = order[starts[b]:ends[b]]
                n = len(e)
                if n == 0:
                    continue
                s0 = int(slot0[g, cl])
                idx_cls[cl][s0:s0 + n] = src[e] - (SPLIT if cl else 0)
                sl = s0 + np.arange(n)
                ks = sl // 128
                part = sl % 128
                segi = seg_of_slot[cl][sl].astype(np.float32)
                dl_arr[cl][part, ks] = segi * 128 + dl[e]
                inv_arr[cl][part, ks] = invv[e]
        wrapped = []
        for cl in (0, 1):
            a = idx_cls[cl]
            w = a.reshape(-1, 16).T.astype(np.int16)
            wrapped.append(np.ascontiguousarray(np.tile(w, (8, 1))))
        per_core.append(dict(idx_lo=wrapped[0], idx_hi=wrapped[1],
                             dl_lo=dl_arr[0], dl_hi=dl_arr[1],
                             inv_lo=inv_arr[0], inv_hi=inv_arr[1]))

    sched = dict(slots_g=slots_g, slot0=slot0, S=S, nchunks=nchunks, NB=NB,
                 chunk_segs=chunk_segs)
    return sched, per_core


def _pack_weights(W, nchunk, H):
    Rr = W.shape[0]
    out = np.zeros((128, Rr * nchunk * H), bf16)
    for r in range(Rr):
        for c in range(nchunk):
            out[:, (r * nchunk + c) * H:(r * nchunk + c + 1) * H] = \
                W[r, c * 128:(c + 1) * 128, :].astype(bf16)
    return out


def _pack_single(Wm, nchunk, H):
    out = np.zeros((128, nchunk * H), bf16)
    for c in range(nchunk):
        out[:, c * H:(c + 1) * H] = Wm[c * 128:(c + 1) * 128, :].astype(bf16)
    return out


def _make_xT(xf, c, width):
    FC = width // 128
    out = np.zeros((128, TILES * FC * 128), bf16)
    base = c * SHARD
    blk = np.zeros((width, TILES * 128), np.float32)
    blk[:, :SHARD] = xf[base:base + SHARD].T
    for t in range(TILES):
        for fc in range(FC):
            out[:, (t * FC + fc) * 128:(t * FC + fc + 1) * 128] = \
                blk[fc * 128:(fc + 1) * 128,
                    t * 128:(t + 1) * 128].astype(bf16)
    return out


# ---------------------------------------------------------------------------
# Device kernel builder (one launch per layer)
# ---------------------------------------------------------------------------
def _build_layer(layer, sched):
    import concourse.bass as bass
    import concourse.mybir as mybir
    from concourse.tile import TileContext
    from concourse.library_config import mlp
    from concourse.library_overlay import lower_extended_insts

    F = IN if layer == 1 else HID
    H = HID if layer == 1 else OUT
    FC = F // 128
    slots_g = sched['slots_g']
    nchunks = sched['nchunks']
    NB = sched['NB']
    chunk_segs = sched['chunk_segs']
    DLCOLS = [max(1, nchunks[0]), max(1, nchunks[1])]

    # ---- plan pass: drive order, first/last MM per group ----
    # events: ('chunk', cl, k) and ('close', g) in emission order
    events = []
    ptr = [0, 0]
    for t in range(TILES):
        for r in range(R):
            g = t * R + r
            for cl in (0, 1):
                while ptr[cl] < nchunks[cl] and \
                        chunk_segs[cl][ptr[cl]] and \
                        chunk_segs[cl][ptr[cl]][0][0] <= g:
                    events.append(('chunk', cl, ptr[cl]))
                    ptr[cl] += 1
            events.append(('close', g))
        events.append(('tile_end', t))
    assert ptr[0] == nchunks[0] and ptr[1] == nchunks[1], \
        f"unconsumed chunks {ptr} vs {nchunks}"
    # first/last (event_index, seg) per group + max simultaneous open psums
    touches = {}
    for ei, ev in enumerate(events):
        if ev[0] == 'chunk':
            cl, k = ev[1], ev[2]
            for (g, si, lo, hi) in chunk_segs[cl][k]:
                touches.setdefault(g, []).append((ei, si))
    first_touch = {g: v[0] for g, v in touches.items()}
    last_touch = {g: v[-1] for g, v in touches.items()}
    open_set = set()
    max_open = 0
    for ei, ev in enumerate(events):
        if ev[0] == 'chunk':
            cl, k = ev[1], ev[2]
            for (g, si, lo, hi) in chunk_segs[cl][k]:
                open_set.add(g)
                max_open = max(max_open, len(open_set))
        elif ev[0] == 'close':
            open_set.discard(ev[1])
    assert max_open <= 4, f"too many simultaneously open groups: {max_open}"

    # batch issuance: first event index that consumes each batch
    first_ev_of_batch = [[], []]
    for cl in (0, 1):
        seen = {}
        for ei, ev in enumerate(events):
            if ev[0] == 'chunk' and ev[1] == cl:
                b = ev[2] // BATCH
                if b not in seen:
                    seen[b] = ei
        first_ev_of_batch[cl] = [seen.get(b, 0) for b in range(NB[cl])]

    nc = bass.Bass()
    dt = mybir.dt
    xsrc = nc.dram_tensor('xsrc', [N, F], dt.bfloat16, kind='ExternalInput')
    idx_lo = nc.dram_tensor('idx_lo', [128, NB[0] * BATCH * 8], dt.int16,
                            kind='ExternalInput')
    idx_hi = nc.dram_tensor('idx_hi', [128, NB[1] * BATCH * 8], dt.int16,
                            kind='ExternalInput')
    dl_lo = nc.dram_tensor('dl_lo', [128, DLCOLS[0]], dt.float32,
                           kind='ExternalInput')
    dl_hi = nc.dram_tensor('dl_hi', [128, DLCOLS[1]], dt.float32,
                           kind='ExternalInput')
    inv_lo = nc.dram_tensor('inv_lo', [128, DLCOLS[0]], dt.float32,
                            kind='ExternalInput')
    inv_hi = nc.dram_tensor('inv_hi', [128, DLCOLS[1]], dt.float32,
                            kind='ExternalInput')
    iota = nc.dram_tensor('iota', [128, MAXSEG * 128], dt.float16,
                          kind='ExternalInput')
    Wsb = nc.dram_tensor('Wsb', [128, R * FC * H], dt.bfloat16,
                         kind='ExternalInput')
    rootsb = nc.dram_tensor('rootsb', [128, FC * H], dt.bfloat16,
                            kind='ExternalInput')
    brow = nc.dram_tensor('brow', [1, H], dt.bfloat16, kind='ExternalInput')
    xT = nc.dram_tensor('xT', [128, TILES * FC * 128], dt.bfloat16,
                        kind='ExternalInput')
    out_dt = dt.bfloat16 if layer == 1 else dt.float32
    yout = nc.dram_tensor('yout', [TILES * 128, H], out_dt,
                          kind='ExternalOutput')

    is_equal = mybir.AluOpType.is_equal
    mult = mybir.AluOpType.mult

    with TileContext(nc) as tc:
        with tc.tile_pool(name='const', bufs=1) as cp, \
             tc.tile_pool(name='glo', bufs=3) as glo, \
             tc.tile_pool(name='ghi', bufs=3) as ghi, \
             tc.tile_pool(name='xtp', bufs=2) as xtp, \
             tc.tile_pool(name='masks', bufs=6) as mp, \
             tc.tile_pool(name='asb', bufs=4) as asb, \
             tc.tile_pool(name='hout', bufs=3) as hp, \
             tc.tile_pool(name='pagg', bufs=5, space='PSUM') as pagg, \
             tc.tile_pool(name='pout', bufs=2, space='PSUM') as pout:

            nc.gpsimd.load_library(mlp)

            il_sb = cp.tile([128, NB[0] * BATCH * 8], dt.int16)
            nc.sync.dma_start(out=il_sb[:], in_=idx_lo[:])
            ih_sb = cp.tile([128, NB[1] * BATCH * 8], dt.int16)
            nc.sync.dma_start(out=ih_sb[:], in_=idx_hi[:])
            dll_sb = cp.tile([128, DLCOLS[0]], dt.float32)
            nc.sync.dma_start(out=dll_sb[:], in_=dl_lo[:])
            dlh_sb = cp.tile([128, DLCOLS[1]], dt.float32)
            nc.sync.dma_start(out=dlh_sb[:], in_=dl_hi[:])
            invl_sb = cp.tile([128, DLCOLS[0]], dt.float32)
            nc.sync.dma_start(out=invl_sb[:], in_=inv_lo[:])
            invh_sb = cp.tile([128, DLCOLS[1]], dt.float32)
            nc.sync.dma_start(out=invh_sb[:], in_=inv_hi[:])
            iota_sb = cp.tile([128, MAXSEG * 128], dt.float16)
            nc.sync.dma_start(out=iota_sb[:], in_=iota[:])
            W_sb = cp.tile([128, R * FC * H], dt.bfloat16)
            nc.sync.dma_start(out=W_sb[:], in_=Wsb[:])
            root_sb = cp.tile([128, FC * H], dt.bfloat16)
            nc.sync.dma_start(out=root_sb[:], in_=rootsb[:])
            b_sb = cp.tile([1, H], dt.bfloat16)
            nc.sync.dma_start(out=b_sb[:], in_=brow[:])
            ones_sb = cp.tile([1, 128], dt.bfloat16)
            nc.vector.memset(ones_sb[:], 1.0)

            idx_sbs = [il_sb, ih_sb]
            dl_sbs = [dll_sb, dlh_sb]
            inv_sbs = [invl_sb, invh_sb]
            srcs = [xsrc[:, :], xsrc[SPLIT:, :]]
            gpools = [glo, ghi]
            gtiles = [[None] * NB[0], [None] * NB[1]]
            next_b = [0, 0]
            # one register per distinct per-call row count (full batches
            # plus each class's trimmed final batch)
            last_nb = [max(1, nchunks[0] - (NB[0] - 1) * BATCH),
                       max(1, nchunks[1] - (NB[1] - 1) * BATCH)]
            nidx_regs = {BATCH * 128: nc.gpsimd.to_reg(BATCH * 128)}
            for nb_ in last_nb:
                if nb_ * 128 not in nidx_regs:
                    nidx_regs[nb_ * 128] = nc.gpsimd.to_reg(nb_ * 128)

            open_psum = {}
            copy_rr = [0]
            opsum_ref = [None]

            def issue_up_to(ei):
                for cl in (0, 1):
                    while (next_b[cl] < NB[cl]
                           and first_ev_of_batch[cl][next_b[cl]]
                           <= ei + 64):
                        k = next_b[cl]
                        nb_ = BATCH if k < NB[cl] - 1 else last_nb[cl]
                        gt = gpools[cl].tile([128, nb_, F], dt.bfloat16,
                                             tag=f'g{cl}b{nb_}')
                        nc.gpsimd.dma_gather(
                            gt[:, :, :], srcs[cl],
                            idx_sbs[cl][:, k * (BATCH * 8):
                                        k * (BATCH * 8) + nb_ * 8],
                            nb_ * 128, nidx_regs[nb_ * 128], F,
                            single_packet=False)
                        gtiles[cl][k] = gt
                        next_b[cl] += 1

            def emit_chunk(ei, cl, k):
                segs = chunk_segs[cl][k]
                if not segs:
                    return
                nseg = len(segs)
                bt = gtiles[cl][k // BATCH]
                col = k % BATCH
                m = mp.tile([128, nseg * 128], dt.bfloat16, tag='m')
                nc.vector.tensor_scalar(
                    out=m[:], in0=iota_sb[:, :nseg * 128],
                    scalar1=dl_sbs[cl][:, k:k + 1],
                    scalar2=inv_sbs[cl][:, k:k + 1],
                    op0=is_equal, op1=mult)
                for (g, si, lo, hi) in segs:
                    ap = open_psum.get(g)
                    if ap is None:
                        ap = pagg.tile([128, 512], dt.float32, tag='a')
                        open_psum[g] = ap
                    first = (first_touch[g] == (ei, si))
                    last = (last_touch[g] == (ei, si))
                    for fc in range(FC):
                        nc.tensor.matmul(
                            out=ap[:, fc * 128:(fc + 1) * 128],
                            lhsT=bt[:, col, fc * 128:(fc + 1) * 128],
                            rhs=m[:, si * 128:(si + 1) * 128],
                            start=(first and fc == 0),
                            stop=(last and fc == FC - 1))

            ei = 0
            for ev in events:
                if ev[0] == 'chunk':
                    issue_up_to(ei)
                    emit_chunk(ei, ev[1], ev[2])
                elif ev[0] == 'close':
                    g = ev[1]
                    t, r = g // R, g % R
                    if r == 0:
                        # open opsum for tile t, root + nothing yet
                        opsum = pout.tile([128, 512], dt.float32, tag='o')
                        opsum_ref[0] = opsum
                        xT_t = xtp.tile([128, FC * 128], dt.bfloat16,
                                        tag='xT')
                        nc.sync.dma_start(
                            out=xT_t[:],
                            in_=xT[:, t * FC * 128:(t + 1) * FC * 128])
                        for fc in range(FC):
                            nc.tensor.matmul(
                                out=opsum[:, :H],
                                lhsT=xT_t[:, fc * 128:(fc + 1) * 128],
                                rhs=root_sb[:, fc * H:(fc + 1) * H],
                                start=(fc == 0), stop=False)
                    opsum = opsum_ref[0]
                    ap = open_psum.pop(g, None)
                    if ap is not None:
                        aggT = asb.tile([128, FC * 128], dt.bfloat16,
                                        tag='at')
                        nc.scalar.activation(
                            out=aggT[:], in_=ap[:, :FC * 128],
                            func=mybir.ActivationFunctionType.Copy)
                        copy_rr[0] += 1
                        for fc in range(FC):
                            nc.tensor.matmul(
                                out=opsum[:, :H],
                                lhsT=aggT[:, fc * 128:(fc + 1) * 128],
                                rhs=W_sb[:, (r * FC + fc) * H:
                                         (r * FC + fc + 1) * H],
                                start=False, stop=False)
                elif ev[0] == 'tile_end':
                    t = ev[1]
                    opsum = opsum_ref[0]
                    nc.tensor.matmul(
                        out=opsum[:, :H], lhsT=ones_sb[:], rhs=b_sb[:],
                        start=False, stop=True)
                    if layer == 1:
                        h_t = hp.tile([128, H], dt.bfloat16, tag='ht')
                        nc.scalar.activation(
                            out=h_t[:], in_=opsum[:, :H],
                            func=mybir.ActivationFunctionType.Relu)
                        nc.sync.dma_start(
                            out=yout[t * 128:(t + 1) * 128, :], in_=h_t[:])
                    else:
                        nrm2 = hp.tile([128, 1], dt.float32, tag='n2')
                        sq = hp.tile([128, OUT], dt.float32, tag='sq')
                        nc.scalar.activation(
                            out=sq[:], in_=opsum[:, :H],
                            func=mybir.ActivationFunctionType.Square,
                            accum_out=nrm2[:])
                        srt = hp.tile([128, 1], dt.float32, tag='srt')
                        nc.scalar.activation(
                            out=srt[:], in_=nrm2[:],
                            func=mybir.ActivationFunctionType.Sqrt)
                        nc.vector.tensor_scalar_max(srt[:], srt[:], 1e-12)
                        rcp = hp.tile([128, 1], dt.float32, tag='rcp')
                        nc.vector.reciprocal(rcp[:], srt[:])
                        o_t = hp.tile([128, OUT], dt.float32, tag='ot')
                        nc.scalar.activation(
                            out=o_t[:], in_=opsum[:, :H],
                            func=mybir.ActivationFunctionType.Copy,
                            scale=rcp[:])
                        nc.sync.dma_start(
                            out=yout[t * 128:(t + 1) * 128, :], in_=o_t[:])
                ei += 1
    lower_extended_insts(nc)
    return nc


def _run(nc, in_maps, trace=False):
    from concourse import bass_utils
    res = bass_utils.run_bass_kernel_spmd(
        nc, in_maps, core_ids=list(range(NCORES)), trace=trace)
    if trace:
        _last_traced[0] = res
    return res


# ---------------------------------------------------------------------------
# Entry point
# ---------------------------------------------------------------------------
def kernel(x, W1, root1, b1, W2, root2, b2, src, dst, edge_type,
           _trace=None):
    _install_tilefix()
    _install_ntff_hook()

    x = np.asarray(x, np.float32)
    sched, per_core = _host_prep(src, dst, edge_type)

    # iota value at column j equals j, compared against dl' = si*128 + dl;
    # fp16 keeps integers up to 2048 exact at full (2-byte) DVE rate.
    iota_np = np.ascontiguousarray(np.broadcast_to(
        np.arange(MAXSEG * 128, dtype=np.float32),
        (128, MAXSEG * 128))).astype(np.float16)
    x_bf = np.ascontiguousarray(x.astype(bf16))
    W1p = _pack_weights(np.asarray(W1, np.float32), IN // 128, HID)
    r1p = _pack_single(np.asarray(root1, np.float32), IN // 128, HID)
    b1p = np.asarray(b1, np.float32)[None, :].astype(bf16)
    W2p = _pack_weights(np.asarray(W2, np.float32), HID // 128, OUT)
    r2p = _pack_single(np.asarray(root2, np.float32), HID // 128, OUT)
    b2p = np.asarray(b2, np.float32)[None, :].astype(bf16)

    def maps_for(c, Wp, rp, bp, xtab, xTc):
        pc = per_core[c]
        return dict(
            xsrc=xtab, idx_lo=pc['idx_lo'], idx_hi=pc['idx_hi'],
            dl_lo=pc['dl_lo'], dl_hi=pc['dl_hi'],
            inv_lo=pc['inv_lo'], inv_hi=pc['inv_hi'],
            iota=iota_np, Wsb=Wp, rootsb=rp, brow=bp, xT=xTc)

    # ---- layer 1 ----
    nc1 = _build_layer(1, sched)
    in_maps1 = [maps_for(c, W1p, r1p, b1p, x_bf, _make_xT(x, c, IN))
                for c in range(NCORES)]
    res1 = _run(nc1, in_maps1, trace=(_trace in ('l1', 'l1_0')))
    _pending_trace['l1'] = res1.exec_time_ns

    h_tab = np.empty((N, HID), bf16)
    for c in range(NCORES):
        h_tab[c * SHARD:(c + 1) * SHARD] = res1.results[c]['yout'][:SHARD]
    h_f32 = h_tab.astype(np.float32)

    # ---- layer 2 ----
    nc2 = _build_layer(2, sched)
    in_maps2 = [maps_for(c, W2p, r2p, b2p, h_tab, _make_xT(h_f32, c, HID))
                for c in range(NCORES)]
    res2 = _run(nc2, in_maps2, trace=(_trace in ('l2', 'l2_0')))
    _pending_trace['l2'] = res2.exec_time_ns

    out = np.empty((N, OUT), np.float32)
    for c in range(NCORES):
        out[c * SHARD:(c + 1) * SHARD] = \
            res2.results[c]['yout'][:SHARD].astype(np.float32)
    return out
